# revision 1
# baseline (speedup 1.0000x reference)
"""Bayesian curve filter kernel for Trainium2 (8 NeuronCores, SPMD).

Sharding: data-parallel over the 1024 Monte-Carlo samples -> 128 per core
(exactly the SBUF partition count; samples live on partitions).

Device algorithm per core (all fp32):
  1. out1 = curves^T @ R : per-sample curve points / velocity / accel
     [128s, 180] (cols 0-59 pts, 60-119 v_t, 120-179 a_t) per dim d.
  2. speeds / centripetal / braking-interp pipeline on [128, 60] tiles.
  3. Boundary nearest-neighbor (the heavy part), soft-select formulation:
       s1[s,b]   = 2 x.b - |b|^2            (argmax_b s1 == argmin_b d2)
       m[s]      = max_b s1                 (DVE reduce over 2x1000 scores)
       t[b,s]    = m - s1 >= 0              (PE re-emission, [b,s] layout)
       H         = exp(-K t)                (ACT; ~one-hot at the argmin)
       sel[4,s]  = sum_b H_b * (e_b, cnx_b, cny_b, 1)   (PE contraction)
       dist      = (sel0 - px*sel1 - py*sel2) / sel3
  4. Per-sample log-score -> w; partial (sum_s w*curve_s, sum_s w) via a
     final [128,17]x[128,1] matmul -> [17] per core; host sums across the
     8 cores and divides (softmax normalization cancels globally).
"""

import os
import numpy as np
from math import comb

import concourse.bass as bass
import concourse.bacc as bacc
import concourse.mybir as mybir
from concourse import tile
from concourse import bass_utils

F32 = mybir.dt.float32
ALU = mybir.AluOpType
AF = mybir.ActivationFunctionType
AX = mybir.AxisListType

NCORES = 8
S_FULL = 1024
SC = 128          # samples per core
P = 60            # points per curve
NB = 1000         # boundary points per boundary
NBP = 1024        # padded
ORD = 7           # bezier order
BETA_SPEED = 0.1
MAX_CA = 19.6
NSEG = 19         # interp segments (20 knots)

_cache = {}


def _diff_mat(n):
    # D [n, n+1]: (D @ c)[k] = c[k+1] - c[k]
    D = np.zeros((n, n + 1), np.float64)
    for k in range(n):
        D[k, k] = -1.0
        D[k, k + 1] = 1.0
    return D


def _build_program(interp_x, interp_dx, interp_m, y0):
    """Builds the bass program. interp constants are baked as immediates."""
    nc = bacc.Bacc("TRN2", target_bir_lowering=False, debug=False, enable_asserts=False)

    # ---- DRAM I/O ----
    d_cv = nc.dram_tensor("cv", [16, SC], F32, kind="ExternalInput").ap()       # curvesT: rows 0-7 x-coefs, 8-15 y
    d_cf = nc.dram_tensor("cf17", [SC, 17], F32, kind="ExternalInput").ap()     # curves flat + ones col
    d_R = nc.dram_tensor("Rm", [8, 180], F32, kind="ExternalInput").ap()
    d_bA = nc.dram_tensor("bA", [3, 2 * NBP], F32, kind="ExternalInput").ap()   # em1 rhs [2bx;2by;-|b|^2]
    d_bG = nc.dram_tensor("bG", [4, 2 * NBP], F32, kind="ExternalInput").ap()   # em2 lhsT [-2bx;-2by;|b|^2;1]
    d_tb = nc.dram_tensor("tb", [SC, 512], F32, kind="ExternalInput").ap()       # select lhsT chunks [e,cnx,cny,1]
    d_I4 = nc.dram_tensor("I4", [4, 4], F32, kind="ExternalInput").ap()
    d_I128 = nc.dram_tensor("I128", [SC, SC], F32, kind="ExternalInput").ap()
    d_Kv = nc.dram_tensor("Kv", [SC, 1], F32, kind="ExternalInput").ap()  # -K replicated
    d_ones = nc.dram_tensor("ones_row", [1, P * SC], F32, kind="ExternalInput").ap()
    d_out = nc.dram_tensor("out17", [17, 1], F32, kind="ExternalOutput").ap()
    d_diag = nc.dram_tensor("diag", [SC, 8], F32, kind="ExternalOutput").ap()

    NQ = 15  # quads of p (4 p's each -> 512-wide sp blocks)

    with tile.TileContext(nc) as tc:
        with (
            tc.tile_pool(name="cst", bufs=1) as cst,
            tc.tile_pool(name="paug", bufs=1) as paugp,
            tc.tile_pool(name="selc", bufs=1) as selcp,
            tc.tile_pool(name="hbuf", bufs=4) as hbuf,
            tc.tile_pool(name="wk", bufs=3) as wk,
            tc.tile_pool(name="m2", bufs=3) as m2p,
            tc.tile_pool(name="big", bufs=3, space="PSUM") as big,
            tc.tile_pool(name="sml", bufs=2, space="PSUM") as sml,
        ):
            # ---- load constants (replicated at partition bases 0/32/64/96) ----
            cvx = cst.tile([8, SC], F32)
            nc.sync.dma_start(cvx[:], d_cv[0:8, :])
            cvy = cst.tile([8, SC], F32)
            nc.sync.dma_start(cvy[:], d_cv[8:16, :])
            cf = cst.tile([SC, 17], F32)
            nc.sync.dma_start(cf[:], d_cf)
            Rm = cst.tile([8, 180], F32)
            nc.sync.dma_start(Rm[:], d_R)
            bA = cst.tile([99, 2 * NBP], F32)
            bG = cst.tile([100, 2 * NBP], F32)
            I4r = cst.tile([100, 4], F32)
            for j in range(4):
                nc.sync.dma_start(bA[32 * j:32 * j + 3, :], d_bA)
                nc.sync.dma_start(bG[32 * j:32 * j + 4, :], d_bG)
                nc.sync.dma_start(I4r[32 * j:32 * j + 4, :], d_I4)
            tb = cst.tile([SC, 512], F32)
            nc.sync.dma_start(tb[:], d_tb)
            I128 = cst.tile([SC, SC], F32)
            nc.sync.dma_start(I128[:], d_I128)
            Kv = cst.tile([SC, 1], F32)
            nc.sync.dma_start(Kv[:], d_Kv)

            # ---- pts/vel/accel in [s, col] layout ----
            o1x = sml.tile([SC, 180], F32, tag="sm")
            nc.tensor.matmul(o1x[:], cvx[:], Rm[:], start=True, stop=True)
            o1y = sml.tile([SC, 180], F32, tag="sm")
            nc.tensor.matmul(o1y[:], cvy[:], Rm[:], start=True, stop=True)
            ox = cst.tile([SC, 180], F32)
            nc.vector.tensor_copy(ox[:], o1x[:])
            oy = cst.tile([SC, 180], F32)
            nc.vector.tensor_copy(oy[:], o1y[:])

            # ---- pts in [p, s] layout -> paug rows ----
            ptx = sml.tile([P, SC], F32, tag="sm")
            nc.tensor.matmul(ptx[:], Rm[:, 0:P], cvx[:], start=True, stop=True)
            pty = sml.tile([P, SC], F32, tag="sm")
            nc.tensor.matmul(pty[:], Rm[:, 0:P], cvy[:], start=True, stop=True)
            ptxs = cst.tile([P, SC], F32)
            nc.vector.tensor_copy(ptxs[:], ptx[:])
            ptys = cst.tile([P, SC], F32)
            nc.vector.tensor_copy(ptys[:], pty[:])

            pgi = paugp.tile([100, P * SC], F32)  # rows 32j..+3: [px; py; 1; m_in]
            pgo = paugp.tile([100, P * SC], F32)  # rows 32j..+3: [px; py; 1; m_out]
            for pg in (pgi, pgo):
                for j in range(4):
                    nc.sync.dma_start(pg[32 * j:32 * j + 1, :].rearrange("o (p s) -> o p s", p=P), ptxs[:])
                    nc.sync.dma_start(pg[32 * j + 1:32 * j + 2, :].rearrange("o (p s) -> o p s", p=P), ptys[:])
                    nc.sync.dma_start(pg[32 * j + 2:32 * j + 3, :], d_ones)

            # ---- boundary: per-quad pipeline, 4-way row/col tiled matmuls ----
            selc = selcp.tile([4, NQ * 1024], F32)
            dTs = selcp.tile([SC, NQ * 32], F32)
            m2qs = [None] * NQ

            def em1(q):
                m2q = m2p.tile([SC, 8], F32, tag="m2")
                m2qs[q] = m2q
                for j4 in range(4):
                    p = 4 * q + j4
                    s1i = big.tile([SC, NBP], F32, tag="big")
                    s1o = big.tile([SC, NBP], F32, tag="big")
                    for bd, s1 in ((0, s1i), (1, s1o)):
                        for h in range(2):
                            g = 32 * (2 * bd + h)
                            nc.tensor.matmul(
                                s1[:, h * 512:(h + 1) * 512],
                                pgi[g:g + 3, p * SC:(p + 1) * SC],
                                bA[g:g + 3, bd * NBP + h * 512: bd * NBP + (h + 1) * 512],
                                start=True, stop=True, tile_position=(g, 0))
                        nc.vector.tensor_reduce(
                            m2q[:, 2 * j4 + bd: 2 * j4 + bd + 1], s1[:], axis=AX.X, op=ALU.max)
                mT = sml.tile([8, SC], F32, tag="sm")
                nc.tensor.matmul(mT[:], m2q[:], I128[:], start=True, stop=True)
                mTs = wk.tile([8, SC], F32, tag="mts")
                nc.scalar.copy(mTs[:], mT[:])
                for j in range(4):
                    nc.sync.dma_start(
                        pgi[32 * j + 3:32 * j + 4, q * 512:(q + 1) * 512].rearrange("o (j2 s) -> o j2 s", j2=4),
                        mTs[0:8:2, :])
                    nc.sync.dma_start(
                        pgo[32 * j + 3:32 * j + 4, q * 512:(q + 1) * 512].rearrange("o (j2 s) -> o j2 s", j2=4),
                        mTs[1:8:2, :])

            def em2(q):
                for bd in range(2):
                    pg = pgi if bd == 0 else pgo
                    sp = sml.tile([4, 512], F32, tag="sm")
                    for hw in range(4):  # windows of 2 chunks
                        t2 = big.tile([SC, NBP], F32, tag="big")
                        for cc in range(2):
                            c = 2 * hw + cc
                            g = 32 * (c % 4)
                            nc.tensor.matmul(
                                t2[:, cc * 512:(cc + 1) * 512],
                                bG[g:g + 4, bd * NBP + c * SC: bd * NBP + (c + 1) * SC],
                                pg[g:g + 4, q * 512:(q + 1) * 512],
                                start=True, stop=True, tile_position=(g, 0))
                        Ht = hbuf.tile([SC, NBP], F32, tag="h")
                        nc.scalar.activation(Ht[:], t2[:], AF.Exp, scale=Kv[:])
                        for cc in range(2):
                            c = 2 * hw + cc
                            nc.tensor.matmul(
                                sp[:], tb[:, (bd * 8 + c) * 32:(bd * 8 + c) * 32 + 4],
                                Ht[:, cc * 512:(cc + 1) * 512],
                                start=(c == 0), stop=(c == 7))
                    nc.scalar.copy(selc[:, q * 1024 + bd * 512: q * 1024 + (bd + 1) * 512], sp[:])
                dTq = sml.tile([SC, 32], F32, tag="sm")
                for j4 in range(4):
                    for bd in range(2):
                        off = q * 1024 + bd * 512 + j4 * SC
                        nc.tensor.matmul(
                            dTq[:, j4 * 8 + bd * 4: j4 * 8 + (bd + 1) * 4],
                            selc[:, off: off + SC], I4r[0:4, :],
                            start=True, stop=True)
                nc.scalar.copy(dTs[:, q * 32:(q + 1) * 32], dTq[:])

            for q in range(NQ + 1):
                if q < NQ:
                    em1(q)
                if q >= 1:
                    em2(q - 1)

            # ---- speeds / accel pipeline [128, 60] ----
            vx, vy, ax_, ay = (ox[:, 60:120], oy[:, 60:120], ox[:, 120:180], oy[:, 120:180])
            spd2 = wk.tile([SC, P], F32)
            nc.vector.tensor_mul(spd2[:], vx, vx)
            t0 = wk.tile([SC, P], F32)
            nc.vector.tensor_mul(t0[:], vy, vy)
            nc.vector.tensor_add(spd2[:], spd2[:], t0[:])
            spd = wk.tile([SC, P], F32)
            nc.scalar.activation(spd[:], spd2[:], AF.Sqrt)
            rspd = wk.tile([SC, P], F32)
            nc.vector.reciprocal(rspd[:], spd[:])
            adv = wk.tile([SC, P], F32)
            nc.vector.tensor_mul(adv[:], ax_, vx)
            nc.vector.tensor_mul(t0[:], ay, vy)
            nc.vector.tensor_add(adv[:], adv[:], t0[:])
            lin = wk.tile([SC, P], F32)
            nc.vector.tensor_mul(lin[:], adv[:], rspd[:])
            a2 = wk.tile([SC, P], F32)
            nc.vector.tensor_mul(a2[:], ax_, ax_)
            nc.vector.tensor_mul(t0[:], ay, ay)
            nc.vector.tensor_add(a2[:], a2[:], t0[:])
            nc.vector.tensor_mul(t0[:], lin[:], lin[:])
            nc.vector.tensor_sub(a2[:], a2[:], t0[:])  # ca^2 (may be ~-eps)
            camax2 = wk.tile([SC, 1], F32)
            nc.vector.tensor_reduce(camax2[:], a2[:], axis=AX.X, op=ALU.max)
            nc.vector.tensor_scalar_max(camax2[:], camax2[:], 0.0)
            camax = wk.tile([SC, 1], F32)
            nc.scalar.activation(camax[:], camax2[:], AF.Sqrt)

            avg = wk.tile([SC, 1], F32)
            nc.vector.tensor_reduce(avg[:], spd[:], axis=AX.X, op=ALU.add)

            # braking interp: bl = y0 + sum_i m_i * clip(spd - x_i, 0, dx_i)
            bl = wk.tile([SC, P], F32)
            nc.vector.memset(bl[:], float(y0))
            ti = wk.tile([SC, P], F32)
            for i in range(NSEG):
                nc.vector.tensor_scalar(ti[:], spd[:], float(interp_x[i]), 0.0, op0=ALU.subtract, op1=ALU.max)
                nc.vector.tensor_scalar(ti[:], ti[:], float(interp_dx[i]), float(interp_m[i]), op0=ALU.min, op1=ALU.mult)
                nc.vector.tensor_add(bl[:], bl[:], ti[:])
            bv = wk.tile([SC, P], F32)
            nc.vector.tensor_sub(bv[:], lin[:], bl[:])
            worst = wk.tile([SC, 1], F32)
            nc.vector.tensor_reduce(worst[:], bv[:], axis=AX.X, op=ALU.min)
            nc.vector.tensor_scalar_min(worst[:], worst[:], 0.0)

            # ---- phase C: dist + maxes (transposes done per-quad above) ----
            bmax = wk.tile([SC, 1], F32)
            for bd in range(2):
                Se = dTs[:, bd * 4 + 0:480:8]
                Scx = dTs[:, bd * 4 + 1:480:8]
                Scy = dTs[:, bd * 4 + 2:480:8]
                Sn = dTs[:, bd * 4 + 3:480:8]
                n1 = wk.tile([SC, P], F32, tag="d1")
                nc.vector.tensor_mul(n1[:], ox[:, 0:P], Scx)
                n2 = wk.tile([SC, P], F32, tag="d2")
                nc.vector.tensor_mul(n2[:], oy[:, 0:P], Scy)
                nc.vector.tensor_sub(n1[:], Se, n1[:])
                nc.vector.tensor_sub(n1[:], n1[:], n2[:])
                rs = wk.tile([SC, P], F32, tag="d3")
                nc.vector.reciprocal(rs[:], Sn)
                nc.vector.tensor_mul(n1[:], n1[:], rs[:])
                dm = wk.tile([SC, 1], F32, tag="d4")
                nc.vector.tensor_reduce(dm[:], n1[:], axis=AX.X, op=ALU.max)
                if bd == 0:
                    nc.vector.tensor_copy(bmax[:], dm[:])
                else:
                    nc.vector.tensor_max(bmax[:], bmax[:], dm[:])
            nc.vector.tensor_scalar_max(bmax[:], bmax[:], 0.0)

            # ---- per-sample scores -> w ----
            args = wk.tile([SC, 1], F32)
            nc.vector.tensor_scalar(args[:], avg[:], float(BETA_SPEED / P), 0.0, op0=ALU.mult, op1=ALU.add)
            nc.vector.tensor_add(args[:], args[:], worst[:])
            ca_pen = wk.tile([SC, 1], F32)
            nc.vector.tensor_scalar(ca_pen[:], camax[:], float(MAX_CA), 0.0, op0=ALU.subtract, op1=ALU.max)
            nc.vector.tensor_sub(args[:], args[:], ca_pen[:])
            e1 = wk.tile([SC, 1], F32)
            nc.scalar.activation(e1[:], args[:], AF.Exp)
            e2 = wk.tile([SC, 1], F32)
            nc.scalar.activation(e2[:], bmax[:], AF.Exp, scale=-1.0)
            nc.vector.tensor_scalar_max(e2[:], e2[:], 1e-32)
            w = wk.tile([SC, 1], F32)
            nc.vector.tensor_mul(w[:], e1[:], e2[:])

            nc.sync.dma_start(d_diag[:, 0:1], w[:])

            # ---- partial sums ----
            op17 = sml.tile([17, 1], F32, tag="sm")
            nc.tensor.matmul(op17[:], cf[:], w[:], start=True, stop=True)
            o17 = wk.tile([17, 1], F32)
            nc.vector.tensor_copy(o17[:], op17[:])
            nc.sync.dma_start(d_out, o17[:])

    nc.compile()
    return nc


def _host_prep(curve, noise, deltaT, speeds_x, braking_y, bezierM, bezierMd, bezierM2d,
               inner_boundary, inner_normals, outer_boundary, outer_normals):
    f64 = np.float64
    dT = float(deltaT)
    curves = (curve[None].astype(f64) + noise.astype(f64))  # [1024, 8, 2]

    # R [8, 180]
    M = bezierM.astype(f64)
    Md = bezierMd.astype(f64)
    M2d = bezierM2d.astype(f64)
    D1 = _diff_mat(7)
    D1b = _diff_mat(6)[:, :7]
    R = np.zeros((8, 180), f64)
    R[:, 0:60] = M.T
    R[:, 60:120] = (7.0 / dT) * (Md @ D1).T
    R[:, 120:180] = (42.0 / (dT * dT)) * (M2d @ D1b @ D1).T

    # boundary tables
    def btab(bpts, bnrm):
        b = bpts.astype(f64)
        n = bnrm.astype(f64)
        b2 = (b * b).sum(1)
        e = (b * n).sum(1)
        A = np.zeros((3, NBP), f64)
        A[0, :NB] = 2 * b[:, 0]
        A[1, :NB] = 2 * b[:, 1]
        A[2, :NB] = -b2
        A[2, NB:] = -1e30
        G = np.zeros((4, NBP), f64)
        G[0, :NB] = -2 * b[:, 0]
        G[1, :NB] = -2 * b[:, 1]
        G[2, :NB] = b2
        G[2, NB:] = 1e30
        G[3, :NB] = 1.0
        T = np.zeros((NBP, 4), f64)
        T[:NB, 0] = e
        T[:NB, 1] = n[:, 0]
        T[:NB, 2] = n[:, 1]
        T[:NB, 3] = 1.0
        return A, G, T, b2.max()

    Ai, Gi, Ti, m2i = btab(inner_boundary, inner_normals)
    Ao, Go, To, m2o = btab(outer_boundary, outer_normals)
    bA = np.concatenate([Ai, Ao], 1)
    bG = np.concatenate([Gi, Go], 1)
    tbl = np.concatenate([Ti, To], 0)  # [2048, 4]
    tblp = np.zeros((2048, 32), np.float64)
    tblp[:, 0:4] = tbl
    tb_sb = np.ascontiguousarray(tblp.reshape(2, 8, 128, 32).transpose(2, 0, 1, 3).reshape(128, 512))

    Bmax2 = max(m2i, m2o, 1.0)
    cmax = max(float(np.abs(curves).max()), 1.0)
    smax = 2.0 * cmax * np.sqrt(Bmax2) + Bmax2
    K = float(2.0 ** 21 / smax)

    # interp constants
    xs = speeds_x.astype(f64)
    ys = braking_y.astype(f64)
    dx = np.diff(xs)
    dx_safe = np.where(dx > 0, dx, 1.0)
    m = np.where(dx > 0, np.diff(ys) / dx_safe, 0.0)

    # per-core shards
    ins = []
    for c in range(NCORES):
        cs = curves[c * SC:(c + 1) * SC]  # [128, 8, 2]
        cv = np.ascontiguousarray(cs.transpose(2, 1, 0).reshape(16, SC)).astype(np.float32)
        cf17 = np.concatenate([cs.reshape(SC, 16), np.ones((SC, 1))], 1).astype(np.float32)
        ins.append(dict(
            cv=cv, cf17=cf17,
            Rm=R.astype(np.float32), bA=bA.astype(np.float32), bG=bG.astype(np.float32),
            tb=tb_sb.astype(np.float32),
            I4=np.eye(4, dtype=np.float32), I128=np.eye(128, dtype=np.float32),
            Kv=np.full((SC, 1), -K, np.float32),
            ones_row=np.ones((1, SC * P), np.float32),
        ))
    return ins, (xs, dx_safe, m, float(ys[0]), K)


def kernel(curve, noise, deltaT, speeds_x, braking_y, bezierM, bezierMd, bezierM2d,
           inner_boundary, inner_normals, outer_boundary, outer_normals):
    in_maps, (xs, dxs, ms, y0, K) = _host_prep(
        curve, noise, deltaT, speeds_x, braking_y, bezierM, bezierMd, bezierM2d,
        inner_boundary, inner_normals, outer_boundary, outer_normals)

    key = (tuple(np.round(xs, 9)), tuple(np.round(ms, 9)), round(y0, 9))
    if key not in _cache:
        _cache.clear()
        _cache[key] = _build_program(xs, dxs, ms, y0)
    nc = _cache[key]

    res = bass_utils.run_bass_kernel_spmd(nc, in_maps, core_ids=list(range(NCORES)))
    outs = res.results
    num = np.zeros(16, np.float64)
    Z = 0.0
    for c in range(NCORES):
        o = np.asarray(outs[c]["out17"]).reshape(17)
        num += o[:16].astype(np.float64)
        Z += float(o[16])
    return (num / Z).reshape(8, 2).astype(np.float32)


if __name__ == "__main__":
    import reference
    inp = {k: np.asarray(v) for k, v in reference.setup_inputs().items()}
    out = kernel(**inp)
    exp = np.asarray(reference.reference(**reference.setup_inputs()))
    err = np.abs(out - exp).max() / (np.abs(exp).max() + 1e-12)
    print("Relative error:", err)



# revision 7
# speedup vs baseline: 1.2616x; 1.2616x over previous
"""Bayesian curve filter kernel for Trainium2 (8 NeuronCores, SPMD).

Sharding: data-parallel over the 1024 Monte-Carlo samples -> 128 per core.

v2 design (vs v1 baseline at 667us):
  * All heavy matmuls in bf16 with hi/lo split rows: each product a*b is
    computed as ah*bh + ah*bl + al*bh over extra K-rows. Matmul cost is
    output-columns only, so extra rows are free; precision ~2^-18 relative.
    bf16 runs 1 cycle/col (fp32 is 4).
  * em1 (score pass for the max) only computes a PREFIX SUBSAMPLE of
    boundary columns (stride-4 reordered first), since its only consumer is
    the soft-select stabilizer m1.  The softmax sharpness K is chosen on the
    host so that exp(K*(s1-m1)) cannot overflow given a host-computed bound
    on (max - m1).  exp still concentrates on the true argmax since scores
    above m1 get exponentially larger weight.
  * K is baked into the boundary tables (bBig); em1 and em2 read the SAME
    SBUF rows so their scores agree bit-for-bit.
  * Soft-select H computed per [128,1024] tile: exp on ACT engine for most
    (q,bd) pairs, step-indicator on DVE for some, to balance engine load.
  * Payload e_b is centered by per-boundary mean E_bd (restored in phase C)
    so bf16 payload rounding is harmless.
  * 4-way tile_position row-group rotation for em1/t2 matmul concurrency.

Device algorithm per core (all accumulation fp32):
  1. o1 = curves^T @ R : points/velocity/accel [128s, 180] per dim.
  2. speeds / centripetal / braking-interp pipeline on [128, 60] tiles.
  3. Boundary soft-select:
       em1: s1K[s, c] = K*(2 x.b - (|b|^2-C)) on subsample cols (PE)
       m1[s]  = max_c s1K  (DVE reduce per point, both boundaries)
       t2[b, ps] = s1K - m1 (PE re-emission, K=12 rows incl. -m hi/lo)
       H = exp(t2) (ACT) or H = [t2 >= -c] (DVE), bf16
       sel[4, ps] = sum_b H_b * (e'_b, nx_b, ny_b, 1)  (PE, bf16)
       dist = E_bd + (sel0 - px*sel1 - py*sel2)/sel3
  4. Per-sample log-score -> w; partial (sum w*curve, sum w) via [128,17]^T
     @ w -> [17] per core; host sums across cores and divides.
"""

import numpy as np
import ml_dtypes
from math import comb

import concourse.bass as bass
import concourse.bacc as bacc
import concourse.mybir as mybir
from concourse import tile
from concourse import bass_utils

F32 = mybir.dt.float32
BF16 = mybir.dt.bfloat16
ALU = mybir.AluOpType
AF = mybir.ActivationFunctionType
AX = mybir.AxisListType
BF = ml_dtypes.bfloat16

NCORES = 8
S_FULL = 1024
SC = 128          # samples per core
P = 60            # points per curve
NB = 1000         # boundary points per boundary
NBP = 1024        # padded
ORD = 7           # bezier order
BETA_SPEED = 0.1
MAX_CA = 19.6
NSEG = 19         # interp segments (20 knots)
SS = 256          # em1 subsample width (prefix of permuted columns)
IND_C = 4.0       # indicator window halfwidth (exp-arg units)
NQ = 15           # point quads

_cache = {}


def _bf_split(x):
    h = np.asarray(x, np.float32).astype(BF)
    l = (np.asarray(x, np.float32) - h.astype(np.float32)).astype(BF)
    return h, l


def _diff_mat(n):
    D = np.zeros((n, n + 1), np.float64)
    for k in range(n):
        D[k, k] = -1.0
        D[k, k + 1] = 1.0
    return D


def _ind_form(q, bd):
    """True -> this (q,bd) uses DVE step-indicator instead of ACT exp."""
    return q % 4 == 1


def _build_program(interp_x, interp_dx, interp_m, y0, e_in, e_out):
    nc = bacc.Bacc("TRN2", target_bir_lowering=False, debug=False, enable_asserts=False)

    d_cv = nc.dram_tensor("cv", [16, SC], F32, kind="ExternalInput").ap()
    d_cf = nc.dram_tensor("cf17", [SC, 17], F32, kind="ExternalInput").ap()
    d_R = nc.dram_tensor("Rm", [8, 180], F32, kind="ExternalInput").ap()
    d_pgs = nc.dram_tensor("pgs", [8, P * SC], BF16, kind="ExternalInput").ap()
    d_bB = nc.dram_tensor("bB", [12, 2 * NBP], BF16, kind="ExternalInput").ap()
    d_tb = nc.dram_tensor("tb", [SC, 512], BF16, kind="ExternalInput").ap()
    d_I4 = nc.dram_tensor("I4", [4, 4], F32, kind="ExternalInput").ap()
    d_I128n = nc.dram_tensor("I128n", [SC, SC], BF16, kind="ExternalInput").ap()
    d_out = nc.dram_tensor("out17", [17, 1], F32, kind="ExternalOutput").ap()

    with tile.TileContext(nc) as tc:
        with (
            tc.tile_pool(name="cst", bufs=1) as cst,
            tc.tile_pool(name="paug", bufs=1) as paugp,
            tc.tile_pool(name="selc", bufs=1) as selcp,
            tc.tile_pool(name="hbuf", bufs=4) as hbuf,
            tc.tile_pool(name="wk", bufs=3) as wk,
            tc.tile_pool(name="m2", bufs=3) as m2p,
            tc.tile_pool(name="msp", bufs=3) as mspp,
            tc.tile_pool(name="ps1", bufs=2, space="PSUM") as ps1,
            tc.tile_pool(name="pt2", bufs=2, space="PSUM") as pt2,
            tc.tile_pool(name="sml", bufs=2, space="PSUM") as sml,
        ):
            # ---- constants ----
            cvx = cst.tile([8, SC], F32)
            nc.sync.dma_start(cvx[:], d_cv[0:8, :])
            cvy = cst.tile([8, SC], F32)
            nc.sync.dma_start(cvy[:], d_cv[8:16, :])
            cf = cst.tile([SC, 17], F32)
            nc.sync.dma_start(cf[:], d_cf)
            Rm = cst.tile([8, 180], F32)
            nc.sync.dma_start(Rm[:], d_R)
            bB = cst.tile([108, 2 * NBP], BF16)
            I4r = cst.tile([100, 4], F32)
            for j in range(4):
                nc.sync.dma_start(bB[32 * j:32 * j + 12, :], d_bB)
                nc.sync.dma_start(I4r[32 * j:32 * j + 4, :], d_I4)
            tb = cst.tile([SC, 512], BF16)
            nc.sync.dma_start(tb[:], d_tb)
            I128n = cst.tile([SC, SC], BF16)
            nc.sync.dma_start(I128n[:], d_I128n)

            # pg: rows 32j+0..7 static [pxh,pxh,pxl,pyh,pyh,pyl,1,1],
            #     rows 32j+8..11 per-quad [-mh_in,-ml_in,-mh_out,-ml_out]
            pg = paugp.tile([108, P * SC], BF16)
            for j in range(4):
                nc.sync.dma_start(pg[32 * j:32 * j + 8, :], d_pgs)

            # ---- pts/vel/accel in [s, col] layout (fp32) ----
            o1x = sml.tile([SC, 180], F32, tag="sm")
            nc.tensor.matmul(o1x[:], cvx[:], Rm[:], start=True, stop=True)
            o1y = sml.tile([SC, 180], F32, tag="sm")
            nc.tensor.matmul(o1y[:], cvy[:], Rm[:], start=True, stop=True)
            ox = cst.tile([SC, 180], F32)
            nc.vector.tensor_copy(ox[:], o1x[:])
            oy = cst.tile([SC, 180], F32)
            nc.vector.tensor_copy(oy[:], o1y[:])

            # ---- boundary loop ----
            selc = selcp.tile([4, NQ * 1024], F32)
            dTs = selcp.tile([SC, NQ * 32], F32)

            def em1(q):
                m2q = m2p.tile([SC, 8], F32, tag="m2")
                for j4 in range(4):
                    p = 4 * q + j4
                    s1 = ps1.tile([SC, 512], F32, tag="s1")
                    for bd in range(2):
                        g = 0  # BISECT: was 32 * (2 * (p % 2) + bd)
                        nc.tensor.matmul(
                            s1[:, bd * SS:(bd + 1) * SS],
                            pg[g:g + 8, p * SC:(p + 1) * SC],
                            bB[g:g + 8, bd * NBP: bd * NBP + SS],
                            start=True, stop=True, tile_position=(g, 0))
                    nc.vector.tensor_reduce(
                        m2q[:, 2 * j4: 2 * j4 + 2],
                        s1[:].rearrange("s (bd c) -> s bd c", bd=2),
                        axis=AX.X, op=ALU.max)
                # bf16 hi/lo split of m, interleaved cols (j4, src, bd)
                m2hl = mspp.tile([SC, 16], BF16, tag="ms")
                for bd in range(2):
                    nc.scalar.copy(m2hl[:, bd:16:4], m2q[:, bd:8:2])
                    nc.vector.tensor_sub(m2hl[:, 2 + bd:16:4], m2q[:, bd:8:2],
                                         m2hl[:, bd:16:4])
                # transpose to rows [-mh_in, -mh_out, -ml_in, -ml_out] x (j2, s)
                mTall = sml.tile([4, 512], F32, tag="sm")
                for j4 in range(4):
                    nc.tensor.matmul(mTall[:, j4 * SC:(j4 + 1) * SC],
                                     m2hl[:, 4 * j4:4 * j4 + 4], I128n[:],
                                     start=True, stop=True)
                mall = mspp.tile([4, 512], BF16, tag="ma")
                nc.scalar.copy(mall[:], mTall[:])
                for j in range(4):
                    nc.sync.dma_start(
                        pg[32 * j + 8:32 * j + 12, q * 512:(q + 1) * 512], mall[:])

            def em2(q):
                for bd in range(2):
                    sp = sml.tile([4, 512], F32, tag="sm")
                    for hw in range(4):
                        t2 = pt2.tile([SC, NBP], F32, tag="t2")
                        for cc in range(2):
                            c = 2 * hw + cc
                            g = 0  # BISECT: was 32 * (c % 4)
                            nc.tensor.matmul(
                                t2[:, cc * 512:(cc + 1) * 512],
                                bB[g:g + 12, bd * NBP + c * SC: bd * NBP + (c + 1) * SC],
                                pg[g:g + 12, q * 512:(q + 1) * 512],
                                start=True, stop=True, tile_position=(g, 0))
                        Ht = hbuf.tile([SC, NBP], BF16, tag="h")
                        if _ind_form(q, bd):
                            nc.vector.tensor_scalar(
                                Ht[:], t2[:], -IND_C, 0.0,
                                op0=ALU.is_ge, op1=ALU.bypass)
                        else:
                            nc.scalar.activation(Ht[:], t2[:], AF.Exp)
                        for cc in range(2):
                            c = 2 * hw + cc
                            nc.tensor.matmul(
                                sp[:], tb[:, (bd * 8 + c) * 32:(bd * 8 + c) * 32 + 4],
                                Ht[:, cc * 512:(cc + 1) * 512],
                                start=(c == 0), stop=(c == 7))
                    off = q * 1024 + bd * 512
                    if q % 2 == 0:
                        nc.scalar.copy(selc[:, off: off + 512], sp[:])
                    else:
                        nc.vector.tensor_copy(selc[:, off: off + 512], sp[:])
                dTq = sml.tile([SC, 32], F32, tag="sm")
                for j4 in range(4):
                    for bd in range(2):
                        off = q * 1024 + bd * 512 + j4 * SC
                        nc.tensor.matmul(
                            dTq[:, j4 * 8 + bd * 4: j4 * 8 + (bd + 1) * 4],
                            selc[:, off: off + SC], I4r[0:4, :],
                            start=True, stop=True)
                nc.scalar.copy(dTs[:, q * 32:(q + 1) * 32], dTq[:])

            for q in range(NQ + 1):
                if q < NQ:
                    em1(q)
                if q >= 1:
                    em2(q - 1)

            # ---- speeds / accel pipeline [128, 60] ----
            vx, vy, ax_, ay = (ox[:, 60:120], oy[:, 60:120], ox[:, 120:180], oy[:, 120:180])
            spd2 = wk.tile([SC, P], F32)
            nc.vector.tensor_mul(spd2[:], vx, vx)
            t0 = wk.tile([SC, P], F32)
            nc.vector.tensor_mul(t0[:], vy, vy)
            nc.vector.tensor_add(spd2[:], spd2[:], t0[:])
            spd = wk.tile([SC, P], F32)
            nc.scalar.activation(spd[:], spd2[:], AF.Sqrt)
            rspd = wk.tile([SC, P], F32)
            nc.vector.reciprocal(rspd[:], spd[:])
            adv = wk.tile([SC, P], F32)
            nc.vector.tensor_mul(adv[:], ax_, vx)
            nc.vector.tensor_mul(t0[:], ay, vy)
            nc.vector.tensor_add(adv[:], adv[:], t0[:])
            lin = wk.tile([SC, P], F32)
            nc.vector.tensor_mul(lin[:], adv[:], rspd[:])
            a2 = wk.tile([SC, P], F32)
            nc.vector.tensor_mul(a2[:], ax_, ax_)
            nc.vector.tensor_mul(t0[:], ay, ay)
            nc.vector.tensor_add(a2[:], a2[:], t0[:])
            nc.vector.tensor_mul(t0[:], lin[:], lin[:])
            nc.vector.tensor_sub(a2[:], a2[:], t0[:])
            camax2 = wk.tile([SC, 1], F32)
            nc.vector.tensor_reduce(camax2[:], a2[:], axis=AX.X, op=ALU.max)
            nc.vector.tensor_scalar_max(camax2[:], camax2[:], 0.0)
            camax = wk.tile([SC, 1], F32)
            nc.scalar.activation(camax[:], camax2[:], AF.Sqrt)

            avg = wk.tile([SC, 1], F32)
            nc.vector.tensor_reduce(avg[:], spd[:], axis=AX.X, op=ALU.add)

            bl = wk.tile([SC, P], F32)
            nc.vector.memset(bl[:], float(y0))
            ti = wk.tile([SC, P], F32)
            for i in range(NSEG):
                nc.vector.tensor_scalar(ti[:], spd[:], float(interp_x[i]), 0.0, op0=ALU.subtract, op1=ALU.max)
                nc.vector.tensor_scalar(ti[:], ti[:], float(interp_dx[i]), float(interp_m[i]), op0=ALU.min, op1=ALU.mult)
                nc.vector.tensor_add(bl[:], bl[:], ti[:])
            bv = wk.tile([SC, P], F32)
            nc.vector.tensor_sub(bv[:], lin[:], bl[:])
            worst = wk.tile([SC, 1], F32)
            nc.vector.tensor_reduce(worst[:], bv[:], axis=AX.X, op=ALU.min)
            nc.vector.tensor_scalar_min(worst[:], worst[:], 0.0)

            # ---- phase C: dist + maxes ----
            bmax = wk.tile([SC, 1], F32)
            for bd, eb in ((0, e_in), (1, e_out)):
                Se = dTs[:, bd * 4 + 0:480:8]
                Scx = dTs[:, bd * 4 + 1:480:8]
                Scy = dTs[:, bd * 4 + 2:480:8]
                Sn = dTs[:, bd * 4 + 3:480:8]
                n1 = wk.tile([SC, P], F32, tag="d1")
                nc.vector.tensor_mul(n1[:], ox[:, 0:P], Scx)
                n2 = wk.tile([SC, P], F32, tag="d2")
                nc.vector.tensor_mul(n2[:], oy[:, 0:P], Scy)
                nc.vector.tensor_sub(n1[:], Se, n1[:])
                nc.vector.tensor_sub(n1[:], n1[:], n2[:])
                rs = wk.tile([SC, P], F32, tag="d3")
                nc.vector.reciprocal(rs[:], Sn)
                nc.vector.tensor_mul(n1[:], n1[:], rs[:])
                dm = wk.tile([SC, 1], F32, tag="d4")
                nc.vector.tensor_reduce(dm[:], n1[:], axis=AX.X, op=ALU.max)
                nc.vector.tensor_scalar(dm[:], dm[:], float(eb), 0.0, op0=ALU.add, op1=ALU.bypass)
                if bd == 0:
                    nc.vector.tensor_copy(bmax[:], dm[:])
                else:
                    nc.vector.tensor_max(bmax[:], bmax[:], dm[:])
            nc.vector.tensor_scalar_max(bmax[:], bmax[:], 0.0)

            # ---- per-sample scores -> w ----
            args = wk.tile([SC, 1], F32)
            nc.vector.tensor_scalar(args[:], avg[:], float(BETA_SPEED / P), 0.0, op0=ALU.mult, op1=ALU.add)
            nc.vector.tensor_add(args[:], args[:], worst[:])
            ca_pen = wk.tile([SC, 1], F32)
            nc.vector.tensor_scalar(ca_pen[:], camax[:], float(MAX_CA), 0.0, op0=ALU.subtract, op1=ALU.max)
            nc.vector.tensor_sub(args[:], args[:], ca_pen[:])
            e1 = wk.tile([SC, 1], F32)
            nc.scalar.activation(e1[:], args[:], AF.Exp)
            e2 = wk.tile([SC, 1], F32)
            nc.scalar.activation(e2[:], bmax[:], AF.Exp, scale=-1.0)
            nc.vector.tensor_scalar_max(e2[:], e2[:], 1e-32)
            w = wk.tile([SC, 1], F32)
            nc.vector.tensor_mul(w[:], e1[:], e2[:])

            op17 = sml.tile([17, 1], F32, tag="sm")
            nc.tensor.matmul(op17[:], cf[:], w[:], start=True, stop=True)
            o17 = wk.tile([17, 1], F32)
            nc.vector.tensor_copy(o17[:], op17[:])
            nc.sync.dma_start(d_out, o17[:])

    nc.compile()
    return nc


def _host_prep(curve, noise, deltaT, speeds_x, braking_y, bezierM, bezierMd, bezierM2d,
               inner_boundary, inner_normals, outer_boundary, outer_normals):
    f64 = np.float64
    dT = float(deltaT)
    curves = (curve[None].astype(f64) + noise.astype(f64))  # [1024, 8, 2]

    # R [8, 180]
    M = bezierM.astype(f64)
    Md = bezierMd.astype(f64)
    M2d = bezierM2d.astype(f64)
    D1 = _diff_mat(7)
    D1b = _diff_mat(6)[:, :7]
    R = np.zeros((8, 180), f64)
    R[:, 0:60] = M.T
    R[:, 60:120] = (7.0 / dT) * (Md @ D1).T
    R[:, 120:180] = (42.0 / (dT * dT)) * (M2d @ D1b @ D1).T

    # curve points per sample [1024, 60, 2] and bound X on |pt|
    pts = np.einsum('pk,skd->spd', M, curves)
    X = float(np.sqrt((pts ** 2).sum(-1)).max()) * 1.000001

    # boundary prefix-subsample permutation: stride-4 set first
    idx = np.arange(NB)
    perm = np.concatenate([idx[0::4], idx[2::4], idx[1::2]])

    def prep_boundary(bpts, bnrm):
        b = bpts.astype(f64)[perm]
        n = bnrm.astype(f64)[perm]
        b2 = (b * b).sum(1)
        e = (b * n).sum(1)
        C = 0.5 * (b2.max() + b2.min())
        E = float(e.mean())
        # subsample shortfall bound: max_b min over 2 nearest subsample pts
        # of max_{|x|<=X} [2 x.(b - b') - (b2 - b2')]
        Ssub = b[:SS]
        d2s = ((b[:, None, :] - Ssub[None, :, :]) ** 2).sum(-1)  # [NB, SS]
        nn = np.argsort(d2s, axis=1)[:, :2]
        delta = 0.0
        for i in range(NB):
            cands = []
            for k in range(2):
                bp = Ssub[nn[i, k]]
                u = 2.0 * (b[i] - bp)
                a = (bp * bp).sum() - b2[i]
                cands.append((a, u))
            (a1, u1), (a2, u2) = cands
            # max over |x|<=X of min(a1+u1.x, a2+u2.x)
            best = -1e30
            f1 = a1 + X * np.sqrt((u1 * u1).sum())
            x1 = X * u1 / (np.sqrt((u1 * u1).sum()) + 1e-30)
            if a2 + u2 @ x1 >= f1 - 1e-12:
                best = max(best, f1)
            f2 = a2 + X * np.sqrt((u2 * u2).sum())
            x2 = X * u2 / (np.sqrt((u2 * u2).sum()) + 1e-30)
            if a1 + u1 @ x2 >= f2 - 1e-12:
                best = max(best, f2)
            d = u1 - u2
            dn2 = (d * d).sum()
            if dn2 > 1e-20:
                x0 = (a2 - a1) * d / dn2
                r2 = X * X - (x0 * x0).sum()
                if r2 >= 0:
                    th = np.array([-d[1], d[0]]) / np.sqrt(dn2)
                    best = max(best, a1 + u1 @ x0 + np.sqrt(r2) * abs(u1 @ th))
            delta = max(delta, best)
        return b, n, b2, e, C, E, max(delta, 0.0)

    bi, ni, b2i, ei, Ci, Ei, di = prep_boundary(inner_boundary, inner_normals)
    bo, no, b2o, eo, Co, Eo, do = prep_boundary(outer_boundary, outer_normals)

    delta = max(di, do)
    K = 70.0 / (delta + 0.05)
    K = float(2.0 ** np.floor(np.log2(max(K, 0.25))))

    # bBig [12, 2048]: rows [2Kbxh,2Kbxl,2Kbxh, 2Kbyh,2Kbyl,2Kbyh,
    #                        -K(b2-C)h, -K(b2-C)l, din,din, dout,dout]
    bB = np.zeros((12, 2 * NBP), np.float32)
    for bd, (b, b2, C) in enumerate(((bi, b2i, Ci), (bo, b2o, Co))):
        o = bd * NBP
        txh, txl = _bf_split(2.0 * K * b[:, 0])
        tyh, tyl = _bf_split(2.0 * K * b[:, 1])
        b2h, b2l = _bf_split(-K * (b2 - C))
        bB[0, o:o + NB] = txh.astype(np.float32)
        bB[1, o:o + NB] = txl.astype(np.float32)
        bB[2, o:o + NB] = txh.astype(np.float32)
        bB[3, o:o + NB] = tyh.astype(np.float32)
        bB[4, o:o + NB] = tyl.astype(np.float32)
        bB[5, o:o + NB] = tyh.astype(np.float32)
        bB[6, o:o + NB] = b2h.astype(np.float32)
        bB[7, o:o + NB] = b2l.astype(np.float32)
        bB[6, o + NB:o + NBP] = -3e38
        bB[8 + bd, o:o + NBP] = 1.0
        bB[10 + bd, o:o + NBP] = 1.0
    bB_bf = bB.astype(BF)

    # payload tables [2048, 4] -> [128, 512] chunk-blocked, e centered
    tbl = np.zeros((2 * NBP, 4), np.float32)
    for bd, (n, e, E) in enumerate(((ni, ei, Ei), (no, eo, Eo))):
        o = bd * NBP
        tbl[o:o + NB, 0] = (e - E).astype(np.float32)
        tbl[o:o + NB, 1] = n[:, 0].astype(np.float32)
        tbl[o:o + NB, 2] = n[:, 1].astype(np.float32)
        tbl[o:o + NB, 3] = 1.0
    tblp = np.zeros((2 * NBP, 32), np.float32)
    tblp[:, 0:4] = tbl
    tb_sb = np.ascontiguousarray(
        tblp.reshape(2, 8, 128, 32).transpose(2, 0, 1, 3).reshape(128, 512)).astype(BF)

    # interp constants
    xs = speeds_x.astype(f64)
    ys = braking_y.astype(f64)
    dx = np.diff(xs)
    dx_safe = np.where(dx > 0, dx, 1.0)
    m = np.where(dx > 0, np.diff(ys) / dx_safe, 0.0)

    # per-core shards
    ins = []
    for c in range(NCORES):
        cs = curves[c * SC:(c + 1) * SC]          # [128, 8, 2]
        pt = pts[c * SC:(c + 1) * SC]             # [128, 60, 2]
        cv = np.ascontiguousarray(cs.transpose(2, 1, 0).reshape(16, SC)).astype(np.float32)
        cf17 = np.concatenate([cs.reshape(SC, 16), np.ones((SC, 1))], 1).astype(np.float32)
        # pgs [8, 60*128]: rows [pxh,pxh,pxl,pyh,pyh,pyl,1,1], p-major cols
        pxT = np.ascontiguousarray(pt[:, :, 0].T).reshape(-1)  # [60*128]
        pyT = np.ascontiguousarray(pt[:, :, 1].T).reshape(-1)
        pxh, pxl = _bf_split(pxT)
        pyh, pyl = _bf_split(pyT)
        pgs = np.zeros((8, P * SC), np.float32)
        pgs[0] = pgs[1] = pxh.astype(np.float32)
        pgs[2] = pxl.astype(np.float32)
        pgs[3] = pgs[4] = pyh.astype(np.float32)
        pgs[5] = pyl.astype(np.float32)
        pgs[6] = pgs[7] = 1.0
        ins.append(dict(
            cv=cv, cf17=cf17, Rm=R.astype(np.float32),
            pgs=pgs.astype(BF), bB=bB_bf, tb=tb_sb,
            I4=np.eye(4, dtype=np.float32),
            I128n=(-np.eye(128)).astype(BF),
        ))
    meta = (xs, dx_safe, m, float(ys[0]), float(Ei), float(Eo), K)
    return ins, meta


def kernel(curve, noise, deltaT, speeds_x, braking_y, bezierM, bezierMd, bezierM2d,
           inner_boundary, inner_normals, outer_boundary, outer_normals):
    in_maps, meta = _host_prep(
        curve, noise, deltaT, speeds_x, braking_y, bezierM, bezierMd, bezierM2d,
        inner_boundary, inner_normals, outer_boundary, outer_normals)
    xs, dxs, ms, y0, Ei, Eo, K = meta

    key = (tuple(np.round(xs, 9)), tuple(np.round(ms, 9)), round(y0, 9),
           round(Ei, 9), round(Eo, 9))
    if key not in _cache:
        _cache.clear()
        _cache[key] = _build_program(xs, dxs, ms, y0, Ei, Eo)
    nc = _cache[key]

    res = bass_utils.run_bass_kernel_spmd(nc, in_maps, core_ids=list(range(NCORES)))
    outs = res.results
    num = np.zeros(16, np.float64)
    Z = 0.0
    for c in range(NCORES):
        o = np.asarray(outs[c]["out17"]).reshape(17)
        num += o[:16].astype(np.float64)
        Z += float(o[16])
    return (num / Z).reshape(8, 2).astype(np.float32)


if __name__ == "__main__":
    import reference
    inp = {k: np.asarray(v) for k, v in reference.setup_inputs().items()}
    out = kernel(**inp)
    exp = np.asarray(reference.reference(**reference.setup_inputs()))
    err = np.abs(out - exp).max() / (np.abs(exp).max() + 1e-12)
    print("Relative error:", err)


# revision 10
# speedup vs baseline: 1.4548x; 1.1531x over previous
"""Bayesian curve filter kernel for Trainium2 (8 NeuronCores, SPMD).

Sharding: data-parallel over the 1024 Monte-Carlo samples -> 128 per core.

v2 design (vs v1 baseline at 667us):
  * All heavy matmuls in bf16 with hi/lo split rows: each product a*b is
    computed as ah*bh + ah*bl + al*bh over extra K-rows. Matmul cost is
    output-columns only, so extra rows are free; precision ~2^-18 relative.
    bf16 runs 1 cycle/col (fp32 is 4).
  * em1 (score pass for the max) only computes a PREFIX SUBSAMPLE of
    boundary columns (stride-4 reordered first), since its only consumer is
    the soft-select stabilizer m1.  The softmax sharpness K is chosen on the
    host so that exp(K*(s1-m1)) cannot overflow given a host-computed bound
    on (max - m1).  exp still concentrates on the true argmax since scores
    above m1 get exponentially larger weight.
  * K is baked into the boundary tables (bBig); em1 and em2 read the SAME
    SBUF rows so their scores agree bit-for-bit.
  * Soft-select H computed per [128,1024] tile: exp on ACT engine for most
    (q,bd) pairs, step-indicator on DVE for some, to balance engine load.
  * Payload e_b is centered by per-boundary mean E_bd (restored in phase C)
    so bf16 payload rounding is harmless.
  * 4-way tile_position row-group rotation for em1/t2 matmul concurrency.

Device algorithm per core (all accumulation fp32):
  1. o1 = curves^T @ R : points/velocity/accel [128s, 180] per dim.
  2. speeds / centripetal / braking-interp pipeline on [128, 60] tiles.
  3. Boundary soft-select:
       em1: s1K[s, c] = K*(2 x.b - (|b|^2-C)) on subsample cols (PE)
       m1[s]  = max_c s1K  (DVE reduce per point, both boundaries)
       t2[b, ps] = s1K - m1 (PE re-emission, K=12 rows incl. -m hi/lo)
       H = exp(t2) (ACT) or H = [t2 >= -c] (DVE), bf16
       sel[4, ps] = sum_b H_b * (e'_b, nx_b, ny_b, 1)  (PE, bf16)
       dist = E_bd + (sel0 - px*sel1 - py*sel2)/sel3
  4. Per-sample log-score -> w; partial (sum w*curve, sum w) via [128,17]^T
     @ w -> [17] per core; host sums across cores and divides.
"""

import numpy as np
import ml_dtypes
from math import comb

import concourse.bass as bass
import concourse.bacc as bacc
import concourse.mybir as mybir
from concourse import tile
from concourse import bass_utils

F32 = mybir.dt.float32
BF16 = mybir.dt.bfloat16
ALU = mybir.AluOpType
AF = mybir.ActivationFunctionType
AX = mybir.AxisListType
BF = ml_dtypes.bfloat16

NCORES = 8
S_FULL = 1024
SC = 128          # samples per core
P = 60            # points per curve
NB = 1000         # boundary points per boundary
NBP = 1024        # padded
ORD = 7           # bezier order
BETA_SPEED = 0.1
MAX_CA = 19.6
NSEG = 19         # interp segments (20 knots)
SS = 256          # em1 subsample width (prefix of permuted columns)
IND_C = 4.0       # indicator window halfwidth (exp-arg units)
NQ = 15           # point quads

_cache = {}


def _bf_split(x):
    h = np.asarray(x, np.float32).astype(BF)
    l = (np.asarray(x, np.float32) - h.astype(np.float32)).astype(BF)
    return h, l


def _diff_mat(n):
    D = np.zeros((n, n + 1), np.float64)
    for k in range(n):
        D[k, k] = -1.0
        D[k, k + 1] = 1.0
    return D


def _ind_form(q, bd):
    """True -> this (q,bd) uses DVE step-indicator instead of ACT exp."""
    return q % 4 == 1


def _build_program(interp_x, interp_dx, interp_m, y0, e_in, e_out):
    nc = bacc.Bacc("TRN2", target_bir_lowering=False, debug=False, enable_asserts=False)

    d_cv = nc.dram_tensor("cv", [16, SC], F32, kind="ExternalInput").ap()
    d_cf = nc.dram_tensor("cf17", [SC, 17], F32, kind="ExternalInput").ap()
    d_R = nc.dram_tensor("Rm", [8, 180], F32, kind="ExternalInput").ap()
    d_pgs = nc.dram_tensor("pgs", [8, P * SC], BF16, kind="ExternalInput").ap()
    d_bB = nc.dram_tensor("bB", [12, 2 * NBP], BF16, kind="ExternalInput").ap()
    d_tb = nc.dram_tensor("tb", [SC, 512], BF16, kind="ExternalInput").ap()
    d_I4 = nc.dram_tensor("I4", [4, 4], F32, kind="ExternalInput").ap()
    d_I128n = nc.dram_tensor("I128n", [SC, SC], BF16, kind="ExternalInput").ap()
    d_out = nc.dram_tensor("out17", [17, 1], F32, kind="ExternalOutput").ap()

    with tile.TileContext(nc) as tc:
        with (
            tc.tile_pool(name="cst", bufs=1) as cst,
            tc.tile_pool(name="paug", bufs=1) as paugp,
            tc.tile_pool(name="selc", bufs=1) as selcp,
            tc.tile_pool(name="hbuf", bufs=4) as hbuf,
            tc.tile_pool(name="wk", bufs=3) as wk,
            tc.tile_pool(name="m2", bufs=3) as m2p,
            tc.tile_pool(name="msp", bufs=3) as mspp,
            tc.tile_pool(name="ps1", bufs=2, space="PSUM") as ps1,
            tc.tile_pool(name="pt2", bufs=2, space="PSUM") as pt2,
            tc.tile_pool(name="sml", bufs=1, space="PSUM") as sml,
            tc.tile_pool(name="psel", bufs=1, space="PSUM") as psel,
        ):
            # ---- constants ----
            cvx = cst.tile([8, SC], F32)
            nc.sync.dma_start(cvx[:], d_cv[0:8, :])
            cvy = cst.tile([8, SC], F32)
            nc.sync.dma_start(cvy[:], d_cv[8:16, :])
            cf = cst.tile([SC, 17], F32)
            nc.sync.dma_start(cf[:], d_cf)
            Rm = cst.tile([8, 180], F32)
            nc.sync.dma_start(Rm[:], d_R)
            bB = cst.tile([108, 2 * NBP], BF16)
            I4r = cst.tile([100, 4], F32)
            for j in range(4):
                nc.sync.dma_start(bB[32 * j:32 * j + 12, :], d_bB)
                nc.sync.dma_start(I4r[32 * j:32 * j + 4, :], d_I4)
            tb = cst.tile([SC, 512], BF16)
            nc.sync.dma_start(tb[:], d_tb)
            I128n = cst.tile([SC, SC], BF16)
            nc.sync.dma_start(I128n[:], d_I128n)

            # pg: rows 32j+0..7 static [pxh,pxh,pxl,pyh,pyh,pyl,1,1],
            #     rows 32j+8..11 per-quad [-mh_in,-ml_in,-mh_out,-ml_out]
            pg = paugp.tile([108, P * SC], BF16)
            for j in range(4):
                nc.sync.dma_start(pg[32 * j:32 * j + 8, :], d_pgs)

            # ---- pts/vel/accel in [s, col] layout (fp32) ----
            o1x = sml.tile([SC, 180], F32, tag="sm")
            nc.tensor.matmul(o1x[:], cvx[:], Rm[:], start=True, stop=True)
            o1y = sml.tile([SC, 180], F32, tag="sm")
            nc.tensor.matmul(o1y[:], cvy[:], Rm[:], start=True, stop=True)
            ox = cst.tile([SC, 180], F32)
            nc.vector.tensor_copy(ox[:], o1x[:])
            oy = cst.tile([SC, 180], F32)
            nc.vector.tensor_copy(oy[:], o1y[:])

            # ---- boundary loop ----
            selc = selcp.tile([100, NQ * 1024], F32)
            dTs = selcp.tile([SC, NQ * 32], F32)

            # zero the sel psum bank once (partitions between col groups are
            # never written by the col-tiled matmuls; dT multiplies them by 0
            # but boot garbage could be NaN/inf)
            zz = cst.tile([1, 512], BF16)
            nc.vector.memset(zz[:], 0.0)
            spz = psel.tile([100, 512], F32, tag="sp")
            nc.tensor.matmul(spz[:], zz[:, 0:100], zz[:], start=True, stop=True)

            def em1(q):
                m2q = m2p.tile([SC, 8], F32, tag="m2")
                for j4 in range(4):
                    p = 4 * q + j4
                    s1 = ps1.tile([SC, 512], F32, tag="s1")
                    for bd in range(2):
                        g = 0  # row-rot disabled (hangs with bf16)
                        nc.tensor.matmul(
                            s1[:, bd * SS:(bd + 1) * SS],
                            pg[g:g + 8, p * SC:(p + 1) * SC],
                            bB[g:g + 8, bd * NBP: bd * NBP + SS],
                            start=True, stop=True, tile_position=(g, 0))
                    nc.vector.tensor_reduce(
                        m2q[:, 2 * j4: 2 * j4 + 2],
                        s1[:].rearrange("s (bd c) -> s bd c", bd=2),
                        axis=AX.X, op=ALU.max)
                # bf16 hi/lo split of m, interleaved cols (j4, src, bd)
                m2hl = mspp.tile([SC, 16], BF16, tag="ms")
                for bd in range(2):
                    nc.scalar.copy(m2hl[:, bd:16:4], m2q[:, bd:8:2])
                    nc.vector.tensor_sub(m2hl[:, 2 + bd:16:4], m2q[:, bd:8:2],
                                         m2hl[:, bd:16:4])
                # transpose to rows [-mh_in, -mh_out, -ml_in, -ml_out] x (j2, s)
                mTall = sml.tile([4, 512], F32, tag="sm")
                for j4 in range(4):
                    nc.tensor.matmul(mTall[:, j4 * SC:(j4 + 1) * SC],
                                     m2hl[:, 4 * j4:4 * j4 + 4], I128n[:],
                                     start=True, stop=True)
                mall = mspp.tile([4, 512], BF16, tag="ma")
                nc.scalar.copy(mall[:], mTall[:])
                for j in range(4):
                    nc.sync.dma_start(
                        pg[32 * j + 8:32 * j + 12, q * 512:(q + 1) * 512], mall[:])

            def em2(q):
                for bd in range(2):
                    sp = psel.tile([100, 512], F32, tag="sp")
                    for hw in range(4):
                        t2 = pt2.tile([SC, NBP], F32, tag="t2")
                        for cc in range(2):
                            c = 2 * hw + cc
                            g = 0  # row-rot disabled (hangs with bf16)
                            nc.tensor.matmul(
                                t2[:, cc * 512:(cc + 1) * 512],
                                bB[g:g + 12, bd * NBP + c * SC: bd * NBP + (c + 1) * SC],
                                pg[g:g + 12, q * 512:(q + 1) * 512],
                                start=True, stop=True, tile_position=(g, 0))
                        Ht = hbuf.tile([SC, NBP], BF16, tag="h")
                        if _ind_form(q, bd):
                            nc.vector.tensor_scalar(
                                Ht[:], t2[:], -IND_C, 0.0,
                                op0=ALU.is_ge, op1=ALU.bypass)
                        else:
                            nc.scalar.activation(Ht[:], t2[:], AF.Exp)
                        for cc in range(2):
                            c = 2 * hw + cc
                            cg = 32 * (c % 4)
                            nc.tensor.matmul(
                                sp[cg:cg + 4, :],
                                tb[:, (bd * 8 + c) * 32:(bd * 8 + c) * 32 + 4],
                                Ht[:, cc * 512:(cc + 1) * 512],
                                start=(c < 4), stop=(c >= 4),
                                tile_position=(0, cg))
                    off = q * 1024 + bd * 512
                    if q % 2 == 0:
                        nc.scalar.copy(selc[:, off: off + 512], sp[:])
                    else:
                        nc.vector.tensor_copy(selc[:, off: off + 512], sp[:])
                dTq = sml.tile([SC, 32], F32, tag="sm")
                for j4 in range(4):
                    for bd in range(2):
                        off = q * 1024 + bd * 512 + j4 * SC
                        nc.tensor.matmul(
                            dTq[:, j4 * 8 + bd * 4: j4 * 8 + (bd + 1) * 4],
                            selc[:, off: off + SC], I4r[:],
                            start=True, stop=True)
                nc.scalar.copy(dTs[:, q * 32:(q + 1) * 32], dTq[:])

            for q in range(NQ + 1):
                if q < NQ:
                    em1(q)
                if q >= 1:
                    em2(q - 1)

            # ---- speeds / accel pipeline [128, 60] ----
            vx, vy, ax_, ay = (ox[:, 60:120], oy[:, 60:120], ox[:, 120:180], oy[:, 120:180])
            spd2 = wk.tile([SC, P], F32)
            nc.vector.tensor_mul(spd2[:], vx, vx)
            t0 = wk.tile([SC, P], F32)
            nc.vector.tensor_mul(t0[:], vy, vy)
            nc.vector.tensor_add(spd2[:], spd2[:], t0[:])
            spd = wk.tile([SC, P], F32)
            nc.scalar.activation(spd[:], spd2[:], AF.Sqrt)
            rspd = wk.tile([SC, P], F32)
            nc.vector.reciprocal(rspd[:], spd[:])
            adv = wk.tile([SC, P], F32)
            nc.vector.tensor_mul(adv[:], ax_, vx)
            nc.vector.tensor_mul(t0[:], ay, vy)
            nc.vector.tensor_add(adv[:], adv[:], t0[:])
            lin = wk.tile([SC, P], F32)
            nc.vector.tensor_mul(lin[:], adv[:], rspd[:])
            a2 = wk.tile([SC, P], F32)
            nc.vector.tensor_mul(a2[:], ax_, ax_)
            nc.vector.tensor_mul(t0[:], ay, ay)
            nc.vector.tensor_add(a2[:], a2[:], t0[:])
            nc.vector.tensor_mul(t0[:], lin[:], lin[:])
            nc.vector.tensor_sub(a2[:], a2[:], t0[:])
            camax2 = wk.tile([SC, 1], F32)
            nc.vector.tensor_reduce(camax2[:], a2[:], axis=AX.X, op=ALU.max)
            nc.vector.tensor_scalar_max(camax2[:], camax2[:], 0.0)
            camax = wk.tile([SC, 1], F32)
            nc.scalar.activation(camax[:], camax2[:], AF.Sqrt)

            avg = wk.tile([SC, 1], F32)
            nc.vector.tensor_reduce(avg[:], spd[:], axis=AX.X, op=ALU.add)

            bl = wk.tile([SC, P], F32)
            nc.vector.memset(bl[:], float(y0))
            ti = wk.tile([SC, P], F32)
            for i in range(NSEG):
                nc.vector.tensor_scalar(ti[:], spd[:], float(interp_x[i]), 0.0, op0=ALU.subtract, op1=ALU.max)
                nc.vector.tensor_scalar(ti[:], ti[:], float(interp_dx[i]), float(interp_m[i]), op0=ALU.min, op1=ALU.mult)
                nc.vector.tensor_add(bl[:], bl[:], ti[:])
            bv = wk.tile([SC, P], F32)
            nc.vector.tensor_sub(bv[:], lin[:], bl[:])
            worst = wk.tile([SC, 1], F32)
            nc.vector.tensor_reduce(worst[:], bv[:], axis=AX.X, op=ALU.min)
            nc.vector.tensor_scalar_min(worst[:], worst[:], 0.0)

            # ---- phase C: dist + maxes ----
            bmax = wk.tile([SC, 1], F32)
            for bd, eb in ((0, e_in), (1, e_out)):
                Se = dTs[:, bd * 4 + 0:480:8]
                Scx = dTs[:, bd * 4 + 1:480:8]
                Scy = dTs[:, bd * 4 + 2:480:8]
                Sn = dTs[:, bd * 4 + 3:480:8]
                n1 = wk.tile([SC, P], F32, tag="d1")
                nc.vector.tensor_mul(n1[:], ox[:, 0:P], Scx)
                n2 = wk.tile([SC, P], F32, tag="d2")
                nc.vector.tensor_mul(n2[:], oy[:, 0:P], Scy)
                nc.vector.tensor_sub(n1[:], Se, n1[:])
                nc.vector.tensor_sub(n1[:], n1[:], n2[:])
                rs = wk.tile([SC, P], F32, tag="d3")
                nc.vector.reciprocal(rs[:], Sn)
                nc.vector.tensor_mul(n1[:], n1[:], rs[:])
                dm = wk.tile([SC, 1], F32, tag="d4")
                nc.vector.tensor_reduce(dm[:], n1[:], axis=AX.X, op=ALU.max)
                nc.vector.tensor_scalar(dm[:], dm[:], float(eb), 0.0, op0=ALU.add, op1=ALU.bypass)
                if bd == 0:
                    nc.vector.tensor_copy(bmax[:], dm[:])
                else:
                    nc.vector.tensor_max(bmax[:], bmax[:], dm[:])
            nc.vector.tensor_scalar_max(bmax[:], bmax[:], 0.0)

            # ---- per-sample scores -> w ----
            args = wk.tile([SC, 1], F32)
            nc.vector.tensor_scalar(args[:], avg[:], float(BETA_SPEED / P), 0.0, op0=ALU.mult, op1=ALU.add)
            nc.vector.tensor_add(args[:], args[:], worst[:])
            ca_pen = wk.tile([SC, 1], F32)
            nc.vector.tensor_scalar(ca_pen[:], camax[:], float(MAX_CA), 0.0, op0=ALU.subtract, op1=ALU.max)
            nc.vector.tensor_sub(args[:], args[:], ca_pen[:])
            e1 = wk.tile([SC, 1], F32)
            nc.scalar.activation(e1[:], args[:], AF.Exp)
            e2 = wk.tile([SC, 1], F32)
            nc.scalar.activation(e2[:], bmax[:], AF.Exp, scale=-1.0)
            nc.vector.tensor_scalar_max(e2[:], e2[:], 1e-32)
            w = wk.tile([SC, 1], F32)
            nc.vector.tensor_mul(w[:], e1[:], e2[:])

            op17 = sml.tile([17, 1], F32, tag="sm")
            nc.tensor.matmul(op17[:], cf[:], w[:], start=True, stop=True)
            o17 = wk.tile([17, 1], F32)
            nc.vector.tensor_copy(o17[:], op17[:])
            nc.sync.dma_start(d_out, o17[:])

    nc.compile()
    return nc


def _host_prep(curve, noise, deltaT, speeds_x, braking_y, bezierM, bezierMd, bezierM2d,
               inner_boundary, inner_normals, outer_boundary, outer_normals):
    f64 = np.float64
    dT = float(deltaT)
    curves = (curve[None].astype(f64) + noise.astype(f64))  # [1024, 8, 2]

    # R [8, 180]
    M = bezierM.astype(f64)
    Md = bezierMd.astype(f64)
    M2d = bezierM2d.astype(f64)
    D1 = _diff_mat(7)
    D1b = _diff_mat(6)[:, :7]
    R = np.zeros((8, 180), f64)
    R[:, 0:60] = M.T
    R[:, 60:120] = (7.0 / dT) * (Md @ D1).T
    R[:, 120:180] = (42.0 / (dT * dT)) * (M2d @ D1b @ D1).T

    # curve points per sample [1024, 60, 2] and bound X on |pt|
    pts = np.einsum('pk,skd->spd', M, curves)
    X = float(np.sqrt((pts ** 2).sum(-1)).max()) * 1.000001

    # boundary prefix-subsample permutation: stride-4 set first
    idx = np.arange(NB)
    perm = np.concatenate([idx[0::4], idx[2::4], idx[1::2]])

    def prep_boundary(bpts, bnrm):
        b = bpts.astype(f64)[perm]
        n = bnrm.astype(f64)[perm]
        b2 = (b * b).sum(1)
        e = (b * n).sum(1)
        C = 0.5 * (b2.max() + b2.min())
        E = float(e.mean())
        # subsample shortfall bound: max_b min over 2 nearest subsample pts
        # of max_{|x|<=X} [2 x.(b - b') - (b2 - b2')]
        Ssub = b[:SS]
        d2s = ((b[:, None, :] - Ssub[None, :, :]) ** 2).sum(-1)  # [NB, SS]
        nn = np.argsort(d2s, axis=1)[:, :2]
        delta = 0.0
        for i in range(NB):
            cands = []
            for k in range(2):
                bp = Ssub[nn[i, k]]
                u = 2.0 * (b[i] - bp)
                a = (bp * bp).sum() - b2[i]
                cands.append((a, u))
            (a1, u1), (a2, u2) = cands
            # max over |x|<=X of min(a1+u1.x, a2+u2.x)
            best = -1e30
            f1 = a1 + X * np.sqrt((u1 * u1).sum())
            x1 = X * u1 / (np.sqrt((u1 * u1).sum()) + 1e-30)
            if a2 + u2 @ x1 >= f1 - 1e-12:
                best = max(best, f1)
            f2 = a2 + X * np.sqrt((u2 * u2).sum())
            x2 = X * u2 / (np.sqrt((u2 * u2).sum()) + 1e-30)
            if a1 + u1 @ x2 >= f2 - 1e-12:
                best = max(best, f2)
            d = u1 - u2
            dn2 = (d * d).sum()
            if dn2 > 1e-20:
                x0 = (a2 - a1) * d / dn2
                r2 = X * X - (x0 * x0).sum()
                if r2 >= 0:
                    th = np.array([-d[1], d[0]]) / np.sqrt(dn2)
                    best = max(best, a1 + u1 @ x0 + np.sqrt(r2) * abs(u1 @ th))
            delta = max(delta, best)
        return b, n, b2, e, C, E, max(delta, 0.0)

    bi, ni, b2i, ei, Ci, Ei, di = prep_boundary(inner_boundary, inner_normals)
    bo, no, b2o, eo, Co, Eo, do = prep_boundary(outer_boundary, outer_normals)

    delta = max(di, do)
    K = 70.0 / (delta + 0.05)
    K = float(2.0 ** np.floor(np.log2(max(K, 0.25))))

    # bBig [12, 2048]: rows [2Kbxh,2Kbxl,2Kbxh, 2Kbyh,2Kbyl,2Kbyh,
    #                        -K(b2-C)h, -K(b2-C)l, din,din, dout,dout]
    bB = np.zeros((12, 2 * NBP), np.float32)
    for bd, (b, b2, C) in enumerate(((bi, b2i, Ci), (bo, b2o, Co))):
        o = bd * NBP
        txh, txl = _bf_split(2.0 * K * b[:, 0])
        tyh, tyl = _bf_split(2.0 * K * b[:, 1])
        b2h, b2l = _bf_split(-K * (b2 - C))
        bB[0, o:o + NB] = txh.astype(np.float32)
        bB[1, o:o + NB] = txl.astype(np.float32)
        bB[2, o:o + NB] = txh.astype(np.float32)
        bB[3, o:o + NB] = tyh.astype(np.float32)
        bB[4, o:o + NB] = tyl.astype(np.float32)
        bB[5, o:o + NB] = tyh.astype(np.float32)
        bB[6, o:o + NB] = b2h.astype(np.float32)
        bB[7, o:o + NB] = b2l.astype(np.float32)
        bB[6, o + NB:o + NBP] = -3e38
        bB[8 + bd, o:o + NBP] = 1.0
        bB[10 + bd, o:o + NBP] = 1.0
    bB_bf = bB.astype(BF)

    # payload tables [2048, 4] -> [128, 512] chunk-blocked, e centered
    tbl = np.zeros((2 * NBP, 4), np.float32)
    for bd, (n, e, E) in enumerate(((ni, ei, Ei), (no, eo, Eo))):
        o = bd * NBP
        tbl[o:o + NB, 0] = (e - E).astype(np.float32)
        tbl[o:o + NB, 1] = n[:, 0].astype(np.float32)
        tbl[o:o + NB, 2] = n[:, 1].astype(np.float32)
        tbl[o:o + NB, 3] = 1.0
    tblp = np.zeros((2 * NBP, 32), np.float32)
    tblp[:, 0:4] = tbl
    tb_sb = np.ascontiguousarray(
        tblp.reshape(2, 8, 128, 32).transpose(2, 0, 1, 3).reshape(128, 512)).astype(BF)

    # interp constants
    xs = speeds_x.astype(f64)
    ys = braking_y.astype(f64)
    dx = np.diff(xs)
    dx_safe = np.where(dx > 0, dx, 1.0)
    m = np.where(dx > 0, np.diff(ys) / dx_safe, 0.0)

    # per-core shards
    ins = []
    for c in range(NCORES):
        cs = curves[c * SC:(c + 1) * SC]          # [128, 8, 2]
        pt = pts[c * SC:(c + 1) * SC]             # [128, 60, 2]
        cv = np.ascontiguousarray(cs.transpose(2, 1, 0).reshape(16, SC)).astype(np.float32)
        cf17 = np.concatenate([cs.reshape(SC, 16), np.ones((SC, 1))], 1).astype(np.float32)
        # pgs [8, 60*128]: rows [pxh,pxh,pxl,pyh,pyh,pyl,1,1], p-major cols
        pxT = np.ascontiguousarray(pt[:, :, 0].T).reshape(-1)  # [60*128]
        pyT = np.ascontiguousarray(pt[:, :, 1].T).reshape(-1)
        pxh, pxl = _bf_split(pxT)
        pyh, pyl = _bf_split(pyT)
        pgs = np.zeros((8, P * SC), np.float32)
        pgs[0] = pgs[1] = pxh.astype(np.float32)
        pgs[2] = pxl.astype(np.float32)
        pgs[3] = pgs[4] = pyh.astype(np.float32)
        pgs[5] = pyl.astype(np.float32)
        pgs[6] = pgs[7] = 1.0
        ins.append(dict(
            cv=cv, cf17=cf17, Rm=R.astype(np.float32),
            pgs=pgs.astype(BF), bB=bB_bf, tb=tb_sb,
            I4=np.eye(4, dtype=np.float32),
            I128n=(-np.eye(128)).astype(BF),
        ))
    meta = (xs, dx_safe, m, float(ys[0]), float(Ei), float(Eo), K)
    return ins, meta


def kernel(curve, noise, deltaT, speeds_x, braking_y, bezierM, bezierMd, bezierM2d,
           inner_boundary, inner_normals, outer_boundary, outer_normals):
    in_maps, meta = _host_prep(
        curve, noise, deltaT, speeds_x, braking_y, bezierM, bezierMd, bezierM2d,
        inner_boundary, inner_normals, outer_boundary, outer_normals)
    xs, dxs, ms, y0, Ei, Eo, K = meta

    key = (tuple(np.round(xs, 9)), tuple(np.round(ms, 9)), round(y0, 9),
           round(Ei, 9), round(Eo, 9))
    if key not in _cache:
        _cache.clear()
        _cache[key] = _build_program(xs, dxs, ms, y0, Ei, Eo)
    nc = _cache[key]

    res = bass_utils.run_bass_kernel_spmd(nc, in_maps, core_ids=list(range(NCORES)))
    outs = res.results
    num = np.zeros(16, np.float64)
    Z = 0.0
    for c in range(NCORES):
        o = np.asarray(outs[c]["out17"]).reshape(17)
        num += o[:16].astype(np.float64)
        Z += float(o[16])
    return (num / Z).reshape(8, 2).astype(np.float32)


if __name__ == "__main__":
    import reference
    inp = {k: np.asarray(v) for k, v in reference.setup_inputs().items()}
    out = kernel(**inp)
    exp = np.asarray(reference.reference(**reference.setup_inputs()))
    err = np.abs(out - exp).max() / (np.abs(exp).max() + 1e-12)
    print("Relative error:", err)


# revision 12
# speedup vs baseline: 1.4620x; 1.0049x over previous
"""Bayesian curve filter kernel for Trainium2 (8 NeuronCores, SPMD).

Sharding: data-parallel over the 1024 Monte-Carlo samples -> 128 per core.

v2 design (vs v1 baseline at 667us):
  * All heavy matmuls in bf16 with hi/lo split rows: each product a*b is
    computed as ah*bh + ah*bl + al*bh over extra K-rows. Matmul cost is
    output-columns only, so extra rows are free; precision ~2^-18 relative.
    bf16 runs 1 cycle/col (fp32 is 4).
  * em1 (score pass for the max) only computes a PREFIX SUBSAMPLE of
    boundary columns (stride-4 reordered first), since its only consumer is
    the soft-select stabilizer m1.  The softmax sharpness K is chosen on the
    host so that exp(K*(s1-m1)) cannot overflow given a host-computed bound
    on (max - m1).  exp still concentrates on the true argmax since scores
    above m1 get exponentially larger weight.
  * K is baked into the boundary tables (bBig); em1 and em2 read the SAME
    SBUF rows so their scores agree bit-for-bit.
  * Soft-select H computed per [128,1024] tile: exp on ACT engine for most
    (q,bd) pairs, step-indicator on DVE for some, to balance engine load.
  * Payload e_b is centered by per-boundary mean E_bd (restored in phase C)
    so bf16 payload rounding is harmless.
  * 4-way tile_position row-group rotation for em1/t2 matmul concurrency.

Device algorithm per core (all accumulation fp32):
  1. o1 = curves^T @ R : points/velocity/accel [128s, 180] per dim.
  2. speeds / centripetal / braking-interp pipeline on [128, 60] tiles.
  3. Boundary soft-select:
       em1: s1K[s, c] = K*(2 x.b - (|b|^2-C)) on subsample cols (PE)
       m1[s]  = max_c s1K  (DVE reduce per point, both boundaries)
       t2[b, ps] = s1K - m1 (PE re-emission, K=12 rows incl. -m hi/lo)
       H = exp(t2) (ACT) or H = [t2 >= -c] (DVE), bf16
       sel[4, ps] = sum_b H_b * (e'_b, nx_b, ny_b, 1)  (PE, bf16)
       dist = E_bd + (sel0 - px*sel1 - py*sel2)/sel3
  4. Per-sample log-score -> w; partial (sum w*curve, sum w) via [128,17]^T
     @ w -> [17] per core; host sums across cores and divides.
"""

import numpy as np
import ml_dtypes
from math import comb

import concourse.bass as bass
import concourse.bacc as bacc
import concourse.mybir as mybir
from concourse import tile
from concourse import bass_utils

F32 = mybir.dt.float32
F32R = mybir.dt.float32r
BF16 = mybir.dt.bfloat16
ALU = mybir.AluOpType
AF = mybir.ActivationFunctionType
AX = mybir.AxisListType
BF = ml_dtypes.bfloat16

NCORES = 8
S_FULL = 1024
SC = 128          # samples per core
P = 60            # points per curve
NB = 1000         # boundary points per boundary
NBP = 1024        # padded
ORD = 7           # bezier order
BETA_SPEED = 0.1
MAX_CA = 19.6
NSEG = 19         # interp segments (20 knots)
SS = 256          # em1 subsample width (prefix of permuted columns)
IND_C = 4.0       # indicator window halfwidth (exp-arg units)
NQ = 15           # point quads

_cache = {}


def _bf_split(x):
    h = np.asarray(x, np.float32).astype(BF)
    l = (np.asarray(x, np.float32) - h.astype(np.float32)).astype(BF)
    return h, l


def _diff_mat(n):
    D = np.zeros((n, n + 1), np.float64)
    for k in range(n):
        D[k, k] = -1.0
        D[k, k + 1] = 1.0
    return D


def _ind_form(q, bd):
    """True -> this (q,bd) uses DVE step-indicator instead of ACT exp."""
    return q % 4 == 1


def _build_program(interp_x, interp_dx, interp_m, y0, e_in, e_out):
    nc = bacc.Bacc("TRN2", target_bir_lowering=False, debug=False, enable_asserts=False)

    d_cv = nc.dram_tensor("cv", [16, SC], F32, kind="ExternalInput").ap()
    d_cf = nc.dram_tensor("cf17", [SC, 17], F32, kind="ExternalInput").ap()
    d_R = nc.dram_tensor("Rm", [8, 180], F32, kind="ExternalInput").ap()
    d_pgs = nc.dram_tensor("pgs", [8, P * SC], BF16, kind="ExternalInput").ap()
    d_bB = nc.dram_tensor("bB", [12, 2 * NBP], BF16, kind="ExternalInput").ap()
    d_tb = nc.dram_tensor("tb", [SC, 512], BF16, kind="ExternalInput").ap()
    d_I4 = nc.dram_tensor("I4", [4, 4], F32, kind="ExternalInput").ap()
    d_I128n = nc.dram_tensor("I128n", [SC, SC], BF16, kind="ExternalInput").ap()
    d_out = nc.dram_tensor("out17", [17, 1], F32, kind="ExternalOutput").ap()

    with tile.TileContext(nc) as tc:
        with (
            tc.tile_pool(name="cst", bufs=1) as cst,
            tc.tile_pool(name="paug", bufs=1) as paugp,
            tc.tile_pool(name="selc", bufs=1) as selcp,
            tc.tile_pool(name="hbuf", bufs=4) as hbuf,
            tc.tile_pool(name="wk", bufs=3) as wk,
            tc.tile_pool(name="m2", bufs=3) as m2p,
            tc.tile_pool(name="msp", bufs=3) as mspp,
            tc.tile_pool(name="ps1", bufs=2, space="PSUM") as ps1,
            tc.tile_pool(name="pt2", bufs=2, space="PSUM") as pt2,
            tc.tile_pool(name="sml", bufs=1, space="PSUM") as sml,
            tc.tile_pool(name="psel", bufs=1, space="PSUM") as psel,
        ):
            # ---- constants ----
            cvx = cst.tile([8, SC], F32)
            nc.sync.dma_start(cvx[:], d_cv[0:8, :])
            cvy = cst.tile([8, SC], F32)
            nc.sync.dma_start(cvy[:], d_cv[8:16, :])
            cf = cst.tile([SC, 17], F32)
            nc.sync.dma_start(cf[:], d_cf)
            Rm = cst.tile([8, 180], F32)
            nc.sync.dma_start(Rm[:], d_R)
            bB = cst.tile([108, 2 * NBP], BF16)
            I4r = cst.tile([100, 4], F32)
            for j in range(4):
                nc.sync.dma_start(bB[32 * j:32 * j + 12, :], d_bB)
                nc.sync.dma_start(I4r[32 * j:32 * j + 4, :], d_I4)
            tb = cst.tile([SC, 512], BF16)
            nc.sync.dma_start(tb[:], d_tb)
            I128n = cst.tile([SC, SC], BF16)
            nc.sync.dma_start(I128n[:], d_I128n)

            # pg: rows 32j+0..7 static [pxh,pxh,pxl,pyh,pyh,pyl,1,1],
            #     rows 32j+8..11 per-quad [-mh_in,-ml_in,-mh_out,-ml_out]
            pg = paugp.tile([108, P * SC], BF16)
            for j in range(4):
                nc.sync.dma_start(pg[32 * j:32 * j + 8, :], d_pgs)

            # ---- pts/vel/accel in [s, col] layout (fp32) ----
            o1x = sml.tile([SC, 180], F32, tag="sm")
            nc.tensor.matmul(o1x[:], cvx[:], Rm[:], start=True, stop=True)
            o1y = sml.tile([SC, 180], F32, tag="sm")
            nc.tensor.matmul(o1y[:], cvy[:], Rm[:], start=True, stop=True)
            ox = cst.tile([SC, 180], F32)
            nc.vector.tensor_copy(ox[:], o1x[:])
            oy = cst.tile([SC, 180], F32)
            nc.vector.tensor_copy(oy[:], o1y[:])

            # ---- boundary loop ----
            selc = selcp.tile([100, NQ * 1024], F32)
            dTs = selcp.tile([SC, NQ * 32], F32)

            # zero the sel psum bank once (partitions between col groups are
            # never written by the col-tiled matmuls; dT multiplies them by 0
            # but boot garbage could be NaN/inf)
            zz = cst.tile([1, 512], BF16)
            nc.vector.memset(zz[:], 0.0)
            spz = psel.tile([100, 512], F32, tag="sp")
            nc.tensor.matmul(spz[:], zz[:, 0:100], zz[:], start=True, stop=True)

            def em1(q):
                m2q = m2p.tile([SC, 8], F32, tag="m2")
                for j4 in range(4):
                    p = 4 * q + j4
                    s1 = ps1.tile([SC, 512], F32, tag="s1")
                    for bd in range(2):
                        g = 0
                        nc.tensor.matmul(
                            s1[:, bd * SS:(bd + 1) * SS],
                            pg[g:g + 8, p * SC:(p + 1) * SC],
                            bB[g:g + 8, bd * NBP: bd * NBP + SS],
                            start=True, stop=True, tile_position=(g, 0))
                    nc.vector.tensor_reduce(
                        m2q[:, 2 * j4: 2 * j4 + 2],
                        s1[:].rearrange("s (bd c) -> s bd c", bd=2),
                        axis=AX.X, op=ALU.max)
                # bf16 hi/lo split of m, interleaved cols (j4, src, bd)
                m2hl = mspp.tile([SC, 16], BF16, tag="ms")
                for bd in range(2):
                    nc.scalar.copy(m2hl[:, bd:16:4], m2q[:, bd:8:2])
                    nc.vector.tensor_sub(m2hl[:, 2 + bd:16:4], m2q[:, bd:8:2],
                                         m2hl[:, bd:16:4])
                # transpose to rows [-mh_in, -mh_out, -ml_in, -ml_out] x (j2, s)
                mTall = sml.tile([4, 512], F32, tag="sm")
                for j4 in range(4):
                    nc.tensor.matmul(mTall[:, j4 * SC:(j4 + 1) * SC],
                                     m2hl[:, 4 * j4:4 * j4 + 4], I128n[:],
                                     start=True, stop=True)
                mall = mspp.tile([4, 512], BF16, tag="ma")
                nc.scalar.copy(mall[:], mTall[:])
                for j in range(4):
                    nc.sync.dma_start(
                        pg[32 * j + 8:32 * j + 12, q * 512:(q + 1) * 512], mall[:])

            def em2(q):
                for bd in range(2):
                    sp = psel.tile([100, 512], F32, tag="sp")
                    for hw in range(4):
                        t2 = pt2.tile([SC, NBP], F32, tag="t2")
                        for cc in range(2):
                            c = 2 * hw + cc
                            g = 0
                            nc.tensor.matmul(
                                t2[:, cc * 512:(cc + 1) * 512],
                                bB[g:g + 12, bd * NBP + c * SC: bd * NBP + (c + 1) * SC],
                                pg[g:g + 12, q * 512:(q + 1) * 512],
                                start=True, stop=True, tile_position=(g, 0))
                        Ht = hbuf.tile([SC, NBP], BF16, tag="h")
                        if _ind_form(q, bd):
                            nc.vector.tensor_scalar(
                                Ht[:], t2[:], -IND_C, 0.0,
                                op0=ALU.is_ge, op1=ALU.bypass)
                        else:
                            nc.scalar.activation(Ht[:], t2[:], AF.Exp)
                        for cc in range(2):
                            c = 2 * hw + cc
                            cg = 32 * (c % 4)
                            nc.tensor.matmul(
                                sp[cg:cg + 4, :],
                                tb[:, (bd * 8 + c) * 32:(bd * 8 + c) * 32 + 4],
                                Ht[:, cc * 512:(cc + 1) * 512],
                                start=(c < 4), stop=(c >= 4),
                                tile_position=(0, cg))
                    off = q * 1024 + bd * 512
                    if q % 2 == 0:
                        nc.scalar.copy(selc[:, off: off + 512], sp[:])
                    else:
                        nc.vector.tensor_copy(selc[:, off: off + 512], sp[:])
                dTq = sml.tile([SC, 32], F32, tag="sm")
                for j4 in range(4):
                    for bd in range(2):
                        off = q * 1024 + bd * 512 + j4 * SC
                        nc.tensor.matmul(
                            dTq[:, j4 * 8 + bd * 4: j4 * 8 + (bd + 1) * 4],
                            selc[:, off: off + SC], I4r[:],
                            start=True, stop=True)
                nc.scalar.copy(dTs[:, q * 32:(q + 1) * 32], dTq[:])

            for q in range(NQ + 1):
                if q < NQ:
                    em1(q)
                if q >= 1:
                    em2(q - 1)

            # ---- speeds / accel pipeline [128, 60] ----
            vx, vy, ax_, ay = (ox[:, 60:120], oy[:, 60:120], ox[:, 120:180], oy[:, 120:180])
            spd2 = wk.tile([SC, P], F32)
            nc.vector.tensor_mul(spd2[:], vx, vx)
            t0 = wk.tile([SC, P], F32)
            nc.vector.tensor_mul(t0[:], vy, vy)
            nc.vector.tensor_add(spd2[:], spd2[:], t0[:])
            spd = wk.tile([SC, P], F32)
            nc.scalar.activation(spd[:], spd2[:], AF.Sqrt)
            rspd = wk.tile([SC, P], F32)
            nc.vector.reciprocal(rspd[:], spd[:])
            adv = wk.tile([SC, P], F32)
            nc.vector.tensor_mul(adv[:], ax_, vx)
            nc.vector.tensor_mul(t0[:], ay, vy)
            nc.vector.tensor_add(adv[:], adv[:], t0[:])
            lin = wk.tile([SC, P], F32)
            nc.vector.tensor_mul(lin[:], adv[:], rspd[:])
            a2 = wk.tile([SC, P], F32)
            nc.vector.tensor_mul(a2[:], ax_, ax_)
            nc.vector.tensor_mul(t0[:], ay, ay)
            nc.vector.tensor_add(a2[:], a2[:], t0[:])
            nc.vector.tensor_mul(t0[:], lin[:], lin[:])
            nc.vector.tensor_sub(a2[:], a2[:], t0[:])
            camax2 = wk.tile([SC, 1], F32)
            nc.vector.tensor_reduce(camax2[:], a2[:], axis=AX.X, op=ALU.max)
            nc.vector.tensor_scalar_max(camax2[:], camax2[:], 0.0)
            camax = wk.tile([SC, 1], F32)
            nc.scalar.activation(camax[:], camax2[:], AF.Sqrt)

            avg = wk.tile([SC, 1], F32)
            nc.vector.tensor_reduce(avg[:], spd[:], axis=AX.X, op=ALU.add)

            bl = wk.tile([SC, P], F32)
            nc.vector.memset(bl[:], float(y0))
            ti = wk.tile([SC, P], F32)
            for i in range(NSEG):
                nc.vector.tensor_scalar(ti[:], spd[:], float(interp_x[i]), 0.0, op0=ALU.subtract, op1=ALU.max)
                nc.vector.tensor_scalar(ti[:], ti[:], float(interp_dx[i]), float(interp_m[i]), op0=ALU.min, op1=ALU.mult)
                nc.vector.tensor_add(bl[:], bl[:], ti[:])
            bv = wk.tile([SC, P], F32)
            nc.vector.tensor_sub(bv[:], lin[:], bl[:])
            worst = wk.tile([SC, 1], F32)
            nc.vector.tensor_reduce(worst[:], bv[:], axis=AX.X, op=ALU.min)
            nc.vector.tensor_scalar_min(worst[:], worst[:], 0.0)

            # ---- phase C: dist + maxes ----
            bmax = wk.tile([SC, 1], F32)
            for bd, eb in ((0, e_in), (1, e_out)):
                Se = dTs[:, bd * 4 + 0:480:8]
                Scx = dTs[:, bd * 4 + 1:480:8]
                Scy = dTs[:, bd * 4 + 2:480:8]
                Sn = dTs[:, bd * 4 + 3:480:8]
                n1 = wk.tile([SC, P], F32, tag="d1")
                nc.vector.tensor_mul(n1[:], ox[:, 0:P], Scx)
                n2 = wk.tile([SC, P], F32, tag="d2")
                nc.vector.tensor_mul(n2[:], oy[:, 0:P], Scy)
                nc.vector.tensor_sub(n1[:], Se, n1[:])
                nc.vector.tensor_sub(n1[:], n1[:], n2[:])
                rs = wk.tile([SC, P], F32, tag="d3")
                nc.vector.reciprocal(rs[:], Sn)
                nc.vector.tensor_mul(n1[:], n1[:], rs[:])
                dm = wk.tile([SC, 1], F32, tag="d4")
                nc.vector.tensor_reduce(dm[:], n1[:], axis=AX.X, op=ALU.max)
                nc.vector.tensor_scalar(dm[:], dm[:], float(eb), 0.0, op0=ALU.add, op1=ALU.bypass)
                if bd == 0:
                    nc.vector.tensor_copy(bmax[:], dm[:])
                else:
                    nc.vector.tensor_max(bmax[:], bmax[:], dm[:])
            nc.vector.tensor_scalar_max(bmax[:], bmax[:], 0.0)

            # ---- per-sample scores -> w ----
            args = wk.tile([SC, 1], F32)
            nc.vector.tensor_scalar(args[:], avg[:], float(BETA_SPEED / P), 0.0, op0=ALU.mult, op1=ALU.add)
            nc.vector.tensor_add(args[:], args[:], worst[:])
            ca_pen = wk.tile([SC, 1], F32)
            nc.vector.tensor_scalar(ca_pen[:], camax[:], float(MAX_CA), 0.0, op0=ALU.subtract, op1=ALU.max)
            nc.vector.tensor_sub(args[:], args[:], ca_pen[:])
            e1 = wk.tile([SC, 1], F32)
            nc.scalar.activation(e1[:], args[:], AF.Exp)
            e2 = wk.tile([SC, 1], F32)
            nc.scalar.activation(e2[:], bmax[:], AF.Exp, scale=-1.0)
            nc.vector.tensor_scalar_max(e2[:], e2[:], 1e-32)
            w = wk.tile([SC, 1], F32)
            nc.vector.tensor_mul(w[:], e1[:], e2[:])

            op17 = sml.tile([17, 1], F32, tag="sm")
            nc.tensor.matmul(op17[:], cf[:], w[:], start=True, stop=True)
            o17 = wk.tile([17, 1], F32)
            nc.vector.tensor_copy(o17[:], op17[:])
            nc.sync.dma_start(d_out, o17[:])

    nc.compile()
    return nc


def _host_prep(curve, noise, deltaT, speeds_x, braking_y, bezierM, bezierMd, bezierM2d,
               inner_boundary, inner_normals, outer_boundary, outer_normals):
    f64 = np.float64
    dT = float(deltaT)
    curves = (curve[None].astype(f64) + noise.astype(f64))  # [1024, 8, 2]

    # R [8, 180]
    M = bezierM.astype(f64)
    Md = bezierMd.astype(f64)
    M2d = bezierM2d.astype(f64)
    D1 = _diff_mat(7)
    D1b = _diff_mat(6)[:, :7]
    R = np.zeros((8, 180), f64)
    R[:, 0:60] = M.T
    R[:, 60:120] = (7.0 / dT) * (Md @ D1).T
    R[:, 120:180] = (42.0 / (dT * dT)) * (M2d @ D1b @ D1).T

    # curve points per sample [1024, 60, 2] and bound X on |pt|
    pts = np.einsum('pk,skd->spd', M, curves)
    X = float(np.sqrt((pts ** 2).sum(-1)).max()) * 1.000001

    # boundary prefix-subsample permutation: stride-4 set first
    idx = np.arange(NB)
    perm = np.concatenate([idx[0::4], idx[2::4], idx[1::2]])

    def prep_boundary(bpts, bnrm):
        b = bpts.astype(f64)[perm]
        n = bnrm.astype(f64)[perm]
        b2 = (b * b).sum(1)
        e = (b * n).sum(1)
        C = 0.5 * (b2.max() + b2.min())
        E = float(e.mean())
        # subsample shortfall bound: max_b min over 2 nearest subsample pts
        # of max_{|x|<=X} [2 x.(b - b') - (b2 - b2')]
        Ssub = b[:SS]
        d2s = ((b[:, None, :] - Ssub[None, :, :]) ** 2).sum(-1)  # [NB, SS]
        nn = np.argsort(d2s, axis=1)[:, :2]
        delta = 0.0
        for i in range(NB):
            cands = []
            for k in range(2):
                bp = Ssub[nn[i, k]]
                u = 2.0 * (b[i] - bp)
                a = (bp * bp).sum() - b2[i]
                cands.append((a, u))
            (a1, u1), (a2, u2) = cands
            # max over |x|<=X of min(a1+u1.x, a2+u2.x)
            best = -1e30
            f1 = a1 + X * np.sqrt((u1 * u1).sum())
            x1 = X * u1 / (np.sqrt((u1 * u1).sum()) + 1e-30)
            if a2 + u2 @ x1 >= f1 - 1e-12:
                best = max(best, f1)
            f2 = a2 + X * np.sqrt((u2 * u2).sum())
            x2 = X * u2 / (np.sqrt((u2 * u2).sum()) + 1e-30)
            if a1 + u1 @ x2 >= f2 - 1e-12:
                best = max(best, f2)
            d = u1 - u2
            dn2 = (d * d).sum()
            if dn2 > 1e-20:
                x0 = (a2 - a1) * d / dn2
                r2 = X * X - (x0 * x0).sum()
                if r2 >= 0:
                    th = np.array([-d[1], d[0]]) / np.sqrt(dn2)
                    best = max(best, a1 + u1 @ x0 + np.sqrt(r2) * abs(u1 @ th))
            delta = max(delta, best)
        return b, n, b2, e, C, E, max(delta, 0.0)

    bi, ni, b2i, ei, Ci, Ei, di = prep_boundary(inner_boundary, inner_normals)
    bo, no, b2o, eo, Co, Eo, do = prep_boundary(outer_boundary, outer_normals)

    delta = max(di, do)
    K = 70.0 / (delta + 0.1)

    # bBig [12, 2048]: rows [2Kbxh,2Kbxl,2Kbxh, 2Kbyh,2Kbyl,2Kbyh,
    #                        -K(b2-C)h, -K(b2-C)l, din, dout, din, dout]
    bB = np.zeros((12, 2 * NBP), np.float32)
    for bd, (b, b2, C) in enumerate(((bi, b2i, Ci), (bo, b2o, Co))):
        o = bd * NBP
        txh, txl = _bf_split(2.0 * K * b[:, 0])
        tyh, tyl = _bf_split(2.0 * K * b[:, 1])
        b2h, b2l = _bf_split(-K * (b2 - C))
        bB[0, o:o + NB] = txh.astype(np.float32)
        bB[1, o:o + NB] = txl.astype(np.float32)
        bB[2, o:o + NB] = txh.astype(np.float32)
        bB[3, o:o + NB] = tyh.astype(np.float32)
        bB[4, o:o + NB] = tyl.astype(np.float32)
        bB[5, o:o + NB] = tyh.astype(np.float32)
        bB[6, o:o + NB] = b2h.astype(np.float32)
        bB[7, o:o + NB] = b2l.astype(np.float32)
        bB[6, o + NB:o + NBP] = -3e38
        bB[8 + bd, o:o + NBP] = 1.0
        bB[10 + bd, o:o + NBP] = 1.0
    bB_bf = bB.astype(BF)

    # payload tables [2048, 4] -> [128, 512] chunk-blocked, e centered
    tbl = np.zeros((2 * NBP, 4), np.float32)
    for bd, (n, e, E) in enumerate(((ni, ei, Ei), (no, eo, Eo))):
        o = bd * NBP
        tbl[o:o + NB, 0] = (e - E).astype(np.float32)
        tbl[o:o + NB, 1] = n[:, 0].astype(np.float32)
        tbl[o:o + NB, 2] = n[:, 1].astype(np.float32)
        tbl[o:o + NB, 3] = 1.0
    tblp = np.zeros((2 * NBP, 32), np.float32)
    tblp[:, 0:4] = tbl
    tb_sb = np.ascontiguousarray(
        tblp.reshape(2, 8, 128, 32).transpose(2, 0, 1, 3).reshape(128, 512)).astype(BF)

    # interp constants
    xs = speeds_x.astype(f64)
    ys = braking_y.astype(f64)
    dx = np.diff(xs)
    dx_safe = np.where(dx > 0, dx, 1.0)
    m = np.where(dx > 0, np.diff(ys) / dx_safe, 0.0)

    # per-core shards
    ins = []
    for c in range(NCORES):
        cs = curves[c * SC:(c + 1) * SC]          # [128, 8, 2]
        pt = pts[c * SC:(c + 1) * SC]             # [128, 60, 2]
        cv = np.ascontiguousarray(cs.transpose(2, 1, 0).reshape(16, SC)).astype(np.float32)
        cf17 = np.concatenate([cs.reshape(SC, 16), np.ones((SC, 1))], 1).astype(np.float32)
        # pgs [8, 60*128]: rows [pxh,pxh,pxl,pyh,pyh,pyl,1,1], p-major cols
        pxT = np.ascontiguousarray(pt[:, :, 0].T).reshape(-1)  # [60*128]
        pyT = np.ascontiguousarray(pt[:, :, 1].T).reshape(-1)
        pxh, pxl = _bf_split(pxT)
        pyh, pyl = _bf_split(pyT)
        pgs = np.zeros((8, P * SC), np.float32)
        pgs[0] = pgs[1] = pxh.astype(np.float32)
        pgs[2] = pxl.astype(np.float32)
        pgs[3] = pgs[4] = pyh.astype(np.float32)
        pgs[5] = pyl.astype(np.float32)
        pgs[6] = pgs[7] = 1.0
        ins.append(dict(
            cv=cv, cf17=cf17, Rm=R.astype(np.float32),
            pgs=pgs.astype(BF), bB=bB_bf, tb=tb_sb,
            I4=np.eye(4, dtype=np.float32),
            I128n=(-np.eye(128)).astype(BF),
        ))
    meta = (xs, dx_safe, m, float(ys[0]), float(Ei), float(Eo), K)
    return ins, meta


def kernel(curve, noise, deltaT, speeds_x, braking_y, bezierM, bezierMd, bezierM2d,
           inner_boundary, inner_normals, outer_boundary, outer_normals):
    in_maps, meta = _host_prep(
        curve, noise, deltaT, speeds_x, braking_y, bezierM, bezierMd, bezierM2d,
        inner_boundary, inner_normals, outer_boundary, outer_normals)
    xs, dxs, ms, y0, Ei, Eo, K = meta

    key = (tuple(np.round(xs, 9)), tuple(np.round(ms, 9)), round(y0, 9),
           round(Ei, 9), round(Eo, 9))
    if key not in _cache:
        _cache.clear()
        _cache[key] = _build_program(xs, dxs, ms, y0, Ei, Eo)
    nc = _cache[key]

    res = bass_utils.run_bass_kernel_spmd(nc, in_maps, core_ids=list(range(NCORES)))
    outs = res.results
    num = np.zeros(16, np.float64)
    Z = 0.0
    for c in range(NCORES):
        o = np.asarray(outs[c]["out17"]).reshape(17)
        num += o[:16].astype(np.float64)
        Z += float(o[16])
    return (num / Z).reshape(8, 2).astype(np.float32)


if __name__ == "__main__":
    import reference
    inp = {k: np.asarray(v) for k, v in reference.setup_inputs().items()}
    out = kernel(**inp)
    exp = np.asarray(reference.reference(**reference.setup_inputs()))
    err = np.abs(out - exp).max() / (np.abs(exp).max() + 1e-12)
    print("Relative error:", err)
